# revision 10
# baseline (speedup 1.0000x reference)
"""GAT message-passing kernel for 8 TRN2 NeuronCores (Bass/Tile).

v5 strategy (dst-sharded, score-in-row, PE/Act-heavy pipeline):
  - Each core owns a contiguous range of destination nodes; the host routes
    each edge to the core owning its destination (edge_index[1]).  Per core,
    8 segments keep compact tables within int16 gather-index space.
  - XP table rows are fp16 [x(64) | sj | si | pad]: phase 1 on-device
    computes per-node scores sj = x.wj, si = x.wi with a single stationary
    W4 matmul over a host-transposed copy of the table (XT2), then writes
    them into the table rows via strided DMA.  A semaphore gates each
    segment's gathers on its score writeback.
  - Phase 2 per chunk of up to 16 tiles (128 slots each, blocks of
    S = deg+1 slots, slot0 = dst):
      dma_gather (4 SWDGE queues rotating, ~2.3ns/row) -> G [128, nb, 128]
      PE: psc = SM @ si  (+ I128 @ sj accumulate)     per-slot raw score
      Act: Lrelu -> Exp  (fp16, lrelu bounds exp input to [-0.09, smax])
      PE: den = BM^T-contract(ex); DVE: rec = 1/den
      PE: recsel = BMT @ rec                           per-slot 1/denominator
      DVE: exr = ex * recsel; exsel = BM x exr         normalized alpha
      PE: per 4-tile group one matmul, shared weights exsel [128, 4m],
          rhs G x-cols [128, 4, 64] -> psU [4m, 4, 64] (3/4 junk bands)
      Act: Relu psU -> fp16 stage; one DMA per chunk -> OUT (junk included)
  - Host extracts the diagonal bands from OUT and assembles the result.
"""
import numpy as np

N_NODES = 100000
HIDDEN = 64
N_CORES = 8
NSEG = 8                 # dst segments per core (int16 index headroom)
LEAKY = 0.01
P = 128
ROW = 128                # fp16 elements per table row (256B, dma_gather)
NBT = 16                 # tiles per gather chunk
GMAX = 4                 # tiles per aggregation group (4m <= 128)
NQUEUES = 4


def _build_layout(edge_src, edge_dst_local, nodes_per_core):
    """Per-core, per-segment compact tables + tile/bucket/chunk structure."""
    ncores = len(edge_src)
    npseg = nodes_per_core // NSEG

    # per (core, seg): sorted edges, per-dst src lists grouped by degree
    seg_info = []  # [core][seg] -> {degree: [(local dst, srcs), ...]}
    for c in range(ncores):
        src, dstl = edge_src[c], edge_dst_local[c]
        order = np.argsort(dstl, kind="stable")
        src, dstl = src[order], dstl[order]
        deg = np.bincount(dstl, minlength=nodes_per_core)
        starts = np.concatenate([[0], np.cumsum(deg)])
        per_seg = []
        for s in range(NSEG):
            lo, hi = s * npseg, (s + 1) * npseg
            per = {}
            for n in range(lo, hi):
                d = int(deg[n])
                if d == 0:
                    continue
                per.setdefault(d, []).append(
                    (n, src[starts[n]:starts[n + 1]]))
            per_seg.append(per)
        seg_info.append(per_seg)

    # shared bucket structure: per segment, union of degrees across cores;
    # n_tiles = max over cores
    program = []  # (seg, d, n_tiles, m)
    for s in range(NSEG):
        all_d = sorted({d for c in range(ncores)
                        for d in seg_info[c][s].keys()})
        for d in all_d:
            if d <= 0 or d > 126:
                raise ValueError(f"unsupported degree {d}")
            S = d + 1
            m = min(P // S, 32)
            maxb = max(len(seg_info[c][s].get(d, [])) for c in range(ncores))
            n_tiles = (maxb + m - 1) // m
            program.append((s, d, n_tiles, m))

    total_tiles = sum(p[2] for p in program)
    total_idx = total_tiles * P
    # groups: per chunk of <=NBT tiles, sub-groups of <=GMAX tiles
    total_groups = 0
    for (s, d, n_tiles, m) in program:
        t = 0
        while t < n_tiles:
            nb = min(NBT, n_tiles - t)
            total_groups += (nb + GMAX - 1) // GMAX
            t += nb

    # per-core: index streams + output extraction maps
    Js = []         # [core] -> int16 [total_idx]
    grows, gcols, gnodes = [], [], []   # per-core extraction indices
    seg_nodes = []  # [core][seg] -> int64 array of global node ids
    for c in range(ncores):
        J = np.zeros(total_idx, dtype=np.int16)
        er, ec, en = [], [], []
        pernodes = []
        i0 = 0
        gi = 0
        base_global = c * nodes_per_core
        for s in range(NSEG):
            srcs_all = [srcs for d, lst in seg_info[c][s].items()
                        for (_, srcs) in lst]
            dsts_all = np.arange(s * npseg, (s + 1) * npseg) + base_global
            allref = np.concatenate(
                [np.concatenate(srcs_all) if srcs_all else
                 np.empty(0, dtype=np.int64), dsts_all])
            nodes = np.unique(allref)
            assert len(nodes) <= 32767, len(nodes)
            pernodes.append(nodes)
            lut = {int(n): j for j, n in enumerate(nodes)}
            for (s2, d, n_tiles, m) in program:
                if s2 != s:
                    continue
                S = d + 1
                lst = seg_info[c][s].get(d, [])
                for bi, (n, srcs) in enumerate(lst):
                    t, b = bi // m, bi % m
                    base = i0 + t * P + b * S
                    J[base] = lut[int(n) + base_global]
                    for e, sv in enumerate(srcs):
                        J[base + 1 + e] = lut[int(sv)]
                # extraction rows for this bucket (chunk/group walk)
                nblocks = len(lst)
                t = 0
                while t < n_tiles:
                    nb = min(NBT, n_tiles - t)
                    k0 = 0
                    while k0 < nb:
                        g = min(GMAX, nb - k0)
                        for k in range(g):
                            tabs = t + k0 + k
                            for b in range(m):
                                bi = tabs * m + b
                                if bi < nblocks:
                                    er.append(gi * P + k * m + b)
                                    ec.append(k)
                                    en.append(lst[bi][0])
                        gi += 1
                        k0 += g
                    t += nb
                i0 += n_tiles * P
        assert gi == total_groups
        Js.append(J)
        grows.append(np.array(er, dtype=np.int64))
        gcols.append(np.array(ec, dtype=np.int64))
        gnodes.append(np.array(en, dtype=np.int64))
        seg_nodes.append(pernodes)
    return (program, total_tiles, total_groups, Js,
            grows, gcols, gnodes, seg_nodes)


def _build_masks(program):
    """Per-bucket masks: BM [P, m] (src slots), SM [P, P] (slot0 select),
    BMT [32, P] (BM transpose)."""
    import ml_dtypes

    keys = sorted({(d, m) for (_, d, _, m) in program})
    bm, sm, bmt, key_idx = [], [], [], {}
    for ki, (d, m) in enumerate(keys):
        S = d + 1
        B = np.zeros((P, m), dtype=np.float32)
        SEL = np.zeros((P, P), dtype=np.float32)
        BT = np.zeros((32, P), dtype=np.float32)
        for p in range(m * S):
            if p % S != 0:
                B[p, p // S] = 1.0
                BT[p // S, p] = 1.0
            SEL[(p // S) * S, p] = 1.0
        bm.append(B)
        sm.append(SEL)
        bmt.append(BT)
        key_idx[(d, m)] = ki
    f16 = ml_dtypes.float16 if hasattr(ml_dtypes, "float16") else np.float16
    bmc = np.concatenate(bm, 1).astype(f16)
    smc = np.concatenate(sm, 1).astype(f16)
    bmtc = np.concatenate(bmt, 1).astype(f16)
    bm_off = np.cumsum([0] + [b.shape[1] for b in bm])
    return bmc, smc, bmtc, bm_off, key_idx


def _pad_rows(r):
    return (r + 1023) // 1024 * 1024


def _build_program(program, total_tiles, total_groups, seg_rows, n_bm_cols,
                   nkeys):
    import os
    import concourse.bass as bass  # noqa: F401
    import concourse.tile as tile
    from concourse import bacc, mybir, library_config
    from concourse.mybir import ActivationFunctionType as AFT

    nosem = os.environ.get("GAT_NOSEM") == "1"
    nowb = os.environ.get("GAT_NOWB") == "1"

    total_idx = total_tiles * P
    pad_rows = [_pad_rows(r) for r in seg_rows]
    seg_base = np.cumsum([0] + pad_rows)
    xt2_cols = [r // 2 for r in pad_rows]
    xt2_base = np.cumsum([0] + xt2_cols)
    F16 = mybir.dt.float16

    nc = bacc.Bacc("TRN2", target_bir_lowering=False,
                   num_swdge_queues=NQUEUES,
                   dynamic_dma_scratch_size=65536)
    XP = nc.dram_tensor("XP", [int(seg_base[-1]), ROW], F16,
                        kind="ExternalInput")
    XT2 = nc.dram_tensor("XT2", [P, int(xt2_base[-1])], F16,
                         kind="ExternalInput")
    IDX = nc.dram_tensor("IDX", [P, total_idx // 16], mybir.dt.int16,
                         kind="ExternalInput")
    W4 = nc.dram_tensor("W4", [P, 4], F16, kind="ExternalInput")
    BM = nc.dram_tensor("BM", [P, n_bm_cols], F16, kind="ExternalInput")
    SM = nc.dram_tensor("SM", [P, P * nkeys], F16, kind="ExternalInput")
    BMT = nc.dram_tensor("BMT", [32, P * nkeys], F16, kind="ExternalInput")
    I128 = nc.dram_tensor("I128", [P, P], F16, kind="ExternalInput")
    OUT = nc.dram_tensor("OUT", [total_groups * P, GMAX * HIDDEN], F16,
                         kind="ExternalOutput")

    STG = 4096  # phase-1 stage columns

    with tile.TileContext(nc) as tc:
        with (
            tc.tile_pool(name="msk", bufs=1) as mskp,
            tc.tile_pool(name="xt2", bufs=2) as xt2p,
            tc.tile_pool(name="scs", bufs=2) as scsp,
            tc.tile_pool(name="g", bufs=6) as gp,
            tc.tile_pool(name="sc", bufs=4) as scp,
            tc.tile_pool(name="es", bufs=3) as esp,
            tc.tile_pool(name="st", bufs=2) as stp,
            tc.tile_pool(name="psS", bufs=2, space="PSUM") as psSp,
            tc.tile_pool(name="psc", bufs=2, space="PSUM") as pscp,
            tc.tile_pool(name="psU", bufs=3, space="PSUM") as psUp,
        ):
            nc.gpsimd.load_library(library_config.mlp)
            sem = nc.alloc_semaphore("scores_done")
            nc.gpsimd.sem_clear(sem)

            bmall = mskp.tile([P, n_bm_cols], F16)
            nc.sync.dma_start(bmall[:], BM[:])
            small = mskp.tile([P, P * nkeys], F16)
            nc.sync.dma_start(small[:], SM[:])
            bmtall = mskp.tile([32, P * nkeys], F16)
            nc.sync.dma_start(bmtall[:], BMT[:])
            i128 = mskp.tile([P, P], F16)
            nc.sync.dma_start(i128[:], I128[:])
            w4 = mskp.tile([P, 4], F16)
            nc.sync.dma_start(w4[:], W4[:])
            jtall = mskp.tile([P, total_idx // 16], mybir.dt.int16)
            nc.sync.dma_start(jtall[:], IDX[:])

            # ---- phase 1: per-segment score computation + writeback ----
            ndma = 0
            seg_wait = []
            for s in range(NSEG):
                cols = xt2_cols[s]
                sc0 = 0
                while sc0 < cols:
                    scols = min(STG, cols - sc0)
                    nblk = scols // P
                    xt = xt2p.tile([P, STG], F16, tag="xt")
                    nc.sync.dma_start(
                        xt[:, :scols],
                        XT2[:, int(xt2_base[s]) + sc0:
                            int(xt2_base[s]) + sc0 + scols])
                    stg = scsp.tile([4, STG], F16, tag="stg")
                    for c0 in range(0, scols, 512):
                        cw = min(512, scols - c0)
                        ps4 = psSp.tile([4, 512], mybir.dt.float32, tag="ps4")
                        nc.tensor.matmul(ps4[:, :cw], lhsT=w4[:],
                                         rhs=xt[:, c0:c0 + cw],
                                         start=True, stop=True)
                        nc.scalar.activation(stg[:, c0:c0 + cw],
                                             ps4[:, :cw], AFT.Copy)
                    # writeback: rows 2*sc0 .. 2*sc0+2*scols of this segment
                    r0 = int(seg_base[s]) + 2 * sc0
                    for h in range(2):
                        for which in range(2):
                            w = 2 * h + which
                            dst = XP[r0:r0 + 2 * scols,
                                     HIDDEN + which:HIDDEN + which + 1
                                     ].rearrange("(i h c) w -> h i (c w)",
                                                 h=2, c=P)[h]
                            if not nowb:
                                inst = nc.sync.dma_start(
                                    dst, stg[w:w + 1, :scols])
                                if not nosem:
                                    inst.then_inc(sem, 16)
                                ndma += 1
                    sc0 += scols
                seg_wait.append(ndma)

            # ---- phase 2 ----
            _, _, _, bm_off, key_idx = _build_masks(program)
            i0 = 0
            gi = 0
            call = 0
            cur_seg = -1
            for (s, d, n_tiles, m) in program:
                S = d + 1
                ki = key_idx[(d, m)]
                if s != cur_seg:
                    if not (nosem or nowb):
                        nc.gpsimd.wait_ge(sem, 16 * seg_wait[s])
                    cur_seg = s
                t = 0
                while t < n_tiles:
                    nb = min(NBT, n_tiles - t)
                    nidx = nb * P
                    G = gp.tile([P, NBT, ROW], F16, tag="G")
                    nc.gpsimd.dma_gather(
                        out_ap=G[:, :nb, :],
                        in_ap=XP[int(seg_base[s]):int(seg_base[s + 1]), :],
                        idxs_ap=jtall[:, i0 // 16:(i0 + nidx) // 16],
                        num_idxs=nidx,
                        num_idxs_reg=nidx,
                        elem_size=ROW,
                        single_packet=False,
                        queue_num=call % NQUEUES,
                    )
                    call += 1
                    # raw scores: psc[p,k] = si[slot0(p)] + sj[p]
                    psc = pscp.tile([P, 3, NBT], mybir.dt.float32, tag="psc")
                    nc.tensor.matmul(
                        psc[:, 0, :nb],
                        lhsT=small[:, ki * P:(ki + 1) * P],
                        rhs=G[:, :nb, HIDDEN + 1],
                        start=True, stop=False)
                    nc.tensor.matmul(
                        psc[:, 0, :nb],
                        lhsT=i128[:],
                        rhs=G[:, :nb, HIDDEN],
                        start=False, stop=True)
                    elr = scp.tile([P, NBT], F16, tag="elr")
                    nc.scalar.activation(elr[:, :nb], psc[:, 0, :nb],
                                         AFT.Lrelu, alpha=LEAKY)
                    ex = scp.tile([P, NBT], F16, tag="ex")
                    nc.scalar.activation(ex[:, :nb], elr[:, :nb], AFT.Exp)
                    # denominators per block, then per-slot reciprocal
                    nc.tensor.matmul(
                        psc[0:m, 1, :nb],
                        lhsT=bmall[:, int(bm_off[ki]):int(bm_off[ki]) + m],
                        rhs=ex[:, :nb],
                        start=True, stop=True)
                    rec = scp.tile([32, NBT], F16, tag="rec")
                    with nc.allow_low_precision(
                            reason="fp16 1/denom; denom in [0.9, 9e4]"):
                        nc.vector.reciprocal(rec[:m, :nb],
                                             psc[0:m, 1, :nb])
                    nc.tensor.matmul(
                        psc[:, 2, :nb],
                        lhsT=bmtall[0:m, ki * P:(ki + 1) * P],
                        rhs=rec[:m, :nb],
                        start=True, stop=True)
                    exr = scp.tile([P, NBT], F16, tag="exr")
                    nc.vector.tensor_mul(exr[:, :nb], ex[:, :nb],
                                         psc[:, 2, :nb])
                    exsel = esp.tile([P, NBT, m], F16, tag="exsel")
                    nc.vector.tensor_mul(
                        exsel[:, :nb, :],
                        bmall[:, int(bm_off[ki]):int(bm_off[ki]) + m
                              ].unsqueeze(1).broadcast_to([P, nb, m]),
                        exr[:, :nb].unsqueeze(2).broadcast_to([P, nb, m]))
                    # aggregation groups
                    stage = stp.tile([P, GMAX, GMAX, HIDDEN], F16, tag="stage")
                    ngc = 0
                    k0 = 0
                    while k0 < nb:
                        g = min(GMAX, nb - k0)
                        psU = psUp.tile([P, GMAX, HIDDEN], mybir.dt.float32,
                                        tag="psU")
                        nc.tensor.matmul(
                            psU[0:g * m, 0:g, :],
                            lhsT=exsel[:, k0:k0 + g, :],
                            rhs=G[:, k0:k0 + g, 0:HIDDEN],
                            start=True, stop=True)
                        nc.scalar.activation(
                            stage[0:g * m, ngc, 0:g, :],
                            psU[0:g * m, 0:g, :], AFT.Relu)
                        ngc += 1
                        k0 += g
                    nc.sync.dma_start(
                        OUT[gi * P:(gi + ngc) * P, :].rearrange(
                            "(g p) e -> p g e", p=P),
                        stage[:, :ngc, :, :])
                    gi += ngc
                    i0 += nidx
                    t += nb
    nc.compile()
    return nc


def _install_profhook():
    """Register the axon NTFF profile hook (missing glue in this container)."""
    import contextlib
    import ctypes
    import sys
    import types

    if "antenv.axon_hooks" in sys.modules:
        return
    try:
        lib = ctypes.CDLL("/opt/axon/libaxon_pjrt.so")
        assert hasattr(lib, "axon_start_nrt_profile")
    except Exception:
        return
    lib.axon_start_nrt_profile.argtypes = [ctypes.POINTER(ctypes.c_int64),
                                           ctypes.c_size_t]
    lib.axon_start_nrt_profile.restype = ctypes.c_int64
    lib.axon_stop_nrt_profile.argtypes = [ctypes.c_char_p]
    lib.axon_stop_nrt_profile.restype = ctypes.c_int64

    @contextlib.contextmanager
    def _hook(output_dir, device_ids):
        import jax

        jax.devices()
        if device_ids:
            ids = (ctypes.c_int64 * len(device_ids))(*device_ids)
            rc = lib.axon_start_nrt_profile(ids, len(device_ids))
        else:
            rc = lib.axon_start_nrt_profile(None, 0)
        if rc != 0:
            raise RuntimeError(f"axon_start_nrt_profile rc={rc}")
        try:
            yield
        finally:
            lib.axon_stop_nrt_profile(str(output_dir).encode())

    mod = types.ModuleType("antenv.axon_hooks")
    mod.get_axon_ntff_profile_hook = lambda: _hook
    mod.set_axon_ntff_profile_hook = lambda h: None
    sys.modules["antenv.axon_hooks"] = mod
    import antenv

    antenv.axon_hooks = mod


def kernel(x, edge_index, w_i, w_j):
    import os
    import ml_dtypes
    from concourse.bass_utils import run_bass_kernel_spmd

    f16 = np.float16
    x = np.asarray(x, dtype=np.float32)
    edge_index = np.asarray(edge_index)
    w_i = np.asarray(w_i, dtype=np.float32)
    w_j = np.asarray(w_j, dtype=np.float32)
    n = x.shape[0]
    assert n == N_NODES and x.shape[1] == HIDDEN
    npc = n // N_CORES

    ej = edge_index[0].astype(np.int64)
    ei = edge_index[1].astype(np.int64)
    core_of = ei // npc
    edge_src, edge_dstl = [], []
    for c in range(N_CORES):
        sel = core_of == c
        edge_src.append(ej[sel])
        edge_dstl.append(ei[sel] - c * npc)

    (program, total_tiles, total_groups, Js,
     grows, gcols, gnodes, seg_nodes) = \
        _build_layout(edge_src, edge_dstl, npc)
    bmc, smc, bmtc, bm_off, key_idx = _build_masks(program)

    # per-segment table sizes uniform across cores (one shared program)
    seg_rows = [max(len(seg_nodes[c][s]) for c in range(N_CORES))
                for s in range(NSEG)]
    pad_rows = [_pad_rows(r) for r in seg_rows]

    W4c = np.zeros((P, 4), dtype=f16)
    W4c[:HIDDEN, 0] = w_j.astype(f16)
    W4c[:HIDDEN, 1] = w_i.astype(f16)
    W4c[HIDDEN:, 2] = w_j.astype(f16)
    W4c[HIDDEN:, 3] = w_i.astype(f16)
    I128c = np.eye(P, dtype=f16)

    nc = _build_program(program, total_tiles, total_groups, seg_rows,
                        bmc.shape[1], len(key_idx))

    x16 = x.astype(f16)
    in_maps = []
    for c in range(N_CORES):
        tabs = []
        xt2s = []
        for s in range(NSEG):
            nodes = seg_nodes[c][s]
            rpad = pad_rows[s]
            t = np.zeros((rpad, ROW), dtype=f16)
            t[:len(nodes), 0:HIDDEN] = x16[nodes]
            tabs.append(t)
            # transposed copy for phase-1: col c2 of block pair i covers
            # rows 256i+c2 (partitions 0:64) and 256i+128+c2 (64:128)
            xr = np.zeros((rpad, HIDDEN), dtype=f16)
            xr[:len(nodes)] = x16[nodes]
            xr = xr.reshape(rpad // 256, 2, P, HIDDEN)
            # [i, h, c2, e] -> [e + 64h, i*128 + c2]
            xt = xr.transpose(1, 3, 0, 2).reshape(2 * HIDDEN, rpad // 2)
            xt2s.append(xt)
        XPc = np.ascontiguousarray(np.concatenate(tabs, 0))
        XT2c = np.ascontiguousarray(np.concatenate(xt2s, 1))
        idxmat = np.ascontiguousarray(Js[c].reshape(-1, 16).T)
        IDXc = np.ascontiguousarray(np.tile(idxmat, (8, 1)))
        in_maps.append({
            "XP": XPc, "XT2": XT2c, "IDX": IDXc,
            "W4": np.ascontiguousarray(W4c),
            "BM": np.ascontiguousarray(bmc),
            "SM": np.ascontiguousarray(smc),
            "BMT": np.ascontiguousarray(bmtc),
            "I128": np.ascontiguousarray(I128c),
        })
    trace = os.environ.get("GAT_TRACE") == "1"
    if trace:
        _install_profhook()
    res = run_bass_kernel_spmd(nc, in_maps, core_ids=list(range(N_CORES)),
                               trace=trace)
    if trace and res.exec_time_ns:
        print(f"HW exec time: {res.exec_time_ns} ns")

    out = np.zeros((n, HIDDEN), dtype=np.float32)
    for c in range(N_CORES):
        ot = res.results[c]["OUT"].reshape(total_groups * P, GMAX, HIDDEN)
        out[c * npc + gnodes[c]] = ot[grows[c], gcols[c], :].astype(
            np.float32)
    return out


# revision 14
# speedup vs baseline: 1.7701x; 1.7701x over previous
"""GAT message-passing kernel for 8 TRN2 NeuronCores (Bass/Tile).

v5.1 strategy (dst-sharded, PE/Act-heavy softmax, no collectives):
  - Each core owns a contiguous range of destination nodes; the host routes
    each edge to the core owning its destination (edge_index[1]).  8 dst
    segments per core keep compact gather tables within int16 index space
    (last segment takes the remainder — v4 dropped the last 4 dsts/core).
  - Edges grouped per destination into blocks of S slots (slot 0 = the
    destination's own row, slots 1..deg = sources, rest point at the
    all-zero sentinel row 0).  Degrees pool into size classes so buckets
    stay large: less cross-core tile padding, fewer (bucket-bound) chunks.
    Table row fp16 [x(64) | flag | pad]; flag=0 on the sentinel masks the
    padded slots out of the softmax.
  - Per chunk of up to 16 tiles (128 slots each):
      dma_gather (4 SWDGE queues rotating, ~2.3ns/row) -> G [128, nb, 128]
      DVE: tmp = G.x * [wj|wi]; sco = reduce -> per-slot sj, si (fp16)
      PE:  psc = SM @ si + I128 @ sj        raw score (slot0 bcast + own sj)
      DVE: lrelu (mul + max);  Act: Exp -> ex fp16   (Act runs ONLY
           Exp/Relu — mixing in Lrelu/Copy thrashed the act table cache)
      DVE: exm = ex * flag;  PE: den = BM-contract(exm);  DVE: rec = 1/den
      PE:  recsel = BMT @ rec -> per-slot 1/denominator
      DVE: exr = exm * recsel; exsel = BM x exr      normalized alpha
      PE:  per <=4-tile group one matmul, stationary exsel [128, g*m],
           moving G.x [128, g, 64] -> psU [g*m, g, 64] (g-1 junk bands —
           junk costs no extra PE columns and the host skips it)
      Act: Relu psU -> fp16 stage
      one DMA per chunk -> OUT, row = base + p*ngc + group: each partition
      writes one contiguous run (the v5 (g p) order scattered 512B bursts)
  - Host extracts the diagonal bands from OUT and assembles the result.
"""
import numpy as np

N_NODES = 100000
HIDDEN = 64
N_CORES = 8
NSEG = 8                 # dst segments per core (int16 index headroom)
LEAKY = 0.01
P = 128
ROW = 128                # fp16 elements per table row (256B, dma_gather)
NBT = 16                 # tiles per gather chunk
GMAX = 4                 # tiles per aggregation group (4m <= 128)
NQUEUES = 4

# block slot-count classes (S = degree+1 rounded up to one of these)
SCLASSES = [2, 3, 4, 5, 6, 7, 8, 9, 10, 11, 12, 14, 16, 18, 20, 23, 26,
            29, 33, 37, 42, 47, 53, 60, 67, 75, 84, 94, 106, 118, 127]


def _sclass(d):
    S = d + 1
    for sc in SCLASSES:
        if sc >= S:
            return sc
    raise ValueError(f"unsupported degree {d}")


def _build_layout(edge_src, edge_dst_local, nodes_per_core):
    """Per-core, per-segment compact tables + class-bucket/chunk structure."""
    ncores = len(edge_src)
    npseg = -(-nodes_per_core // NSEG)  # ceil: last segment holds remainder

    # per (core, seg): {Sclass: [(local dst, srcs), ...]}
    seg_info = []
    for c in range(ncores):
        src, dstl = edge_src[c], edge_dst_local[c]
        order = np.argsort(dstl, kind="stable")
        src, dstl = src[order], dstl[order]
        deg = np.bincount(dstl, minlength=nodes_per_core)
        starts = np.concatenate([[0], np.cumsum(deg)])
        per_seg = []
        for s in range(NSEG):
            lo = s * npseg
            hi = min((s + 1) * npseg, nodes_per_core)
            per = {}
            for n in range(lo, hi):
                d = int(deg[n])
                if d == 0:
                    continue
                per.setdefault(_sclass(d), []).append(
                    (n, src[starts[n]:starts[n + 1]]))
            per_seg.append(per)
        seg_info.append(per_seg)

    # shared bucket structure: n_tiles = max over cores
    program = []  # (seg, S, n_tiles, m)
    for s in range(NSEG):
        all_S = sorted({S for c in range(ncores)
                        for S in seg_info[c][s].keys()})
        for S in all_S:
            m = min(P // S, 32)
            maxb = max(len(seg_info[c][s].get(S, [])) for c in range(ncores))
            n_tiles = (maxb + m - 1) // m
            program.append((s, S, n_tiles, m))

    total_tiles = sum(p[2] for p in program)
    total_idx = total_tiles * P

    # chunk walk (shared): chunk list of (s, S, m, nb, ngc); groups g<=GMAX
    chunks = []
    for (s, S, n_tiles, m) in program:
        t = 0
        while t < n_tiles:
            nb = min(NBT, n_tiles - t)
            ngc = (nb + GMAX - 1) // GMAX
            chunks.append((s, S, m, nb, ngc))
            t += nb
    total_out_rows = sum(P * ch[4] for ch in chunks)

    # per-core: index streams + output extraction maps
    Js = []
    grows, gcols, gnodes = [], [], []
    seg_nodes = []  # [core][seg] -> global node ids (row r+1 = nodes[r])
    for c in range(ncores):
        J = np.zeros(total_idx, dtype=np.int16)
        er, ec, en = [], [], []
        pernodes = []
        i0 = 0
        co = 0  # OUT row base of current chunk
        base_global = c * nodes_per_core
        for s in range(NSEG):
            srcs_all = [srcs for S, lst in seg_info[c][s].items()
                        for (_, srcs) in lst]
            lo = s * npseg
            hi = min((s + 1) * npseg, nodes_per_core)
            dsts_all = np.arange(lo, hi) + base_global
            allref = np.concatenate(
                [np.concatenate(srcs_all) if srcs_all else
                 np.empty(0, dtype=np.int64), dsts_all])
            nodes = np.unique(allref)
            assert len(nodes) <= 32765, len(nodes)
            pernodes.append(nodes)
            # row0 = flag-0 sentinel (class pads), row1 = flag-1 sentinel
            # (pad blocks: den=1, avoids inf/NaN), real rows from 2
            lut = {int(n): j + 2 for j, n in enumerate(nodes)}
            for (s2, S, n_tiles, m) in program:
                if s2 != s:
                    continue
                lst = seg_info[c][s].get(S, [])
                for bi, (n, srcs) in enumerate(lst):
                    t, b = bi // m, bi % m
                    base = i0 + t * P + b * S
                    J[base] = lut[int(n) + base_global]
                    for e, sv in enumerate(srcs):
                        J[base + 1 + e] = lut[int(sv)]
                nblocks = len(lst)
                for bi in range(nblocks, n_tiles * m):
                    t2, b2 = bi // m, bi % m
                    J[i0 + t2 * P + b2 * S + 1] = 1
                t = 0
                while t < n_tiles:
                    nb = min(NBT, n_tiles - t)
                    ngc = (nb + GMAX - 1) // GMAX
                    k0 = 0
                    gslot = 0
                    while k0 < nb:
                        g = min(GMAX, nb - k0)
                        for k in range(g):
                            tabs = t + k0 + k
                            for b in range(m):
                                bi = tabs * m + b
                                if bi < nblocks:
                                    er.append(co + (k * m + b) * ngc + gslot)
                                    ec.append(k)
                                    en.append(lst[bi][0])
                        gslot += 1
                        k0 += g
                    co += P * ngc
                    t += nb
                i0 += n_tiles * P
        assert co == total_out_rows
        Js.append(J)
        grows.append(np.array(er, dtype=np.int64))
        gcols.append(np.array(ec, dtype=np.int64))
        gnodes.append(np.array(en, dtype=np.int64))
        seg_nodes.append(pernodes)
    return (program, total_tiles, total_out_rows, Js,
            grows, gcols, gnodes, seg_nodes)


def _build_masks(program):
    """Per-bucket masks: BM [P, m] (src slots), SM [P, P] (slot0 select),
    BMT [32, P] (BM transpose)."""
    keys = sorted({(S, m) for (_, S, _, m) in program})
    bm, sm, bmt, key_idx = [], [], [], {}
    for ki, (S, m) in enumerate(keys):
        B = np.zeros((P, m), dtype=np.float32)
        SEL = np.zeros((P, P), dtype=np.float32)
        BT = np.zeros((32, P), dtype=np.float32)
        for p in range(m * S):
            if p % S != 0:
                B[p, p // S] = 1.0
                BT[p // S, p] = 1.0
            SEL[(p // S) * S, p] = 1.0
        bm.append(B)
        sm.append(SEL)
        bmt.append(BT)
        key_idx[(S, m)] = ki
    bmc = np.concatenate(bm, 1).astype(np.float16)
    smc = np.concatenate(sm, 1).astype(np.float16)
    bmtc = np.concatenate(bmt, 1).astype(np.float16)
    bm_off = np.cumsum([0] + [b.shape[1] for b in bm])
    return bmc, smc, bmtc, bm_off, key_idx


def _build_program(program, total_tiles, total_out_rows, seg_rows, n_bm_cols,
                   nkeys):
    import concourse.bass as bass  # noqa: F401
    import concourse.tile as tile
    from concourse import bacc, mybir, library_config
    from concourse.mybir import ActivationFunctionType as AFT

    total_idx = total_tiles * P
    seg_base = np.cumsum([0] + list(seg_rows))
    F16 = mybir.dt.float16

    nc = bacc.Bacc("TRN2", target_bir_lowering=False,
                   num_swdge_queues=NQUEUES,
                   dynamic_dma_scratch_size=65536)
    XP = nc.dram_tensor("XP", [int(seg_base[-1]), ROW], F16,
                        kind="ExternalInput")
    IDX = nc.dram_tensor("IDX", [P, total_idx // 16], mybir.dt.int16,
                         kind="ExternalInput")
    W2R = nc.dram_tensor("W2R", [P, 2 * HIDDEN], F16, kind="ExternalInput")
    BM = nc.dram_tensor("BM", [P, n_bm_cols], F16, kind="ExternalInput")
    SM = nc.dram_tensor("SM", [P, P * nkeys], F16, kind="ExternalInput")
    BMT = nc.dram_tensor("BMT", [32, P * nkeys], F16, kind="ExternalInput")
    I128 = nc.dram_tensor("I128", [P, P], F16, kind="ExternalInput")
    OUT = nc.dram_tensor("OUT", [total_out_rows, GMAX * HIDDEN], F16,
                         kind="ExternalOutput")

    _, _, _, bm_off, key_idx = _build_masks(program)

    with tile.TileContext(nc) as tc:
        with (
            tc.tile_pool(name="msk", bufs=1) as mskp,
            tc.tile_pool(name="g", bufs=6) as gp,
            tc.tile_pool(name="tm", bufs=2) as tmp_p,
            tc.tile_pool(name="sc", bufs=4) as scp,
            tc.tile_pool(name="es", bufs=3) as esp,
            tc.tile_pool(name="st", bufs=3) as stp,
            tc.tile_pool(name="psc", bufs=2, space="PSUM") as pscp,
            tc.tile_pool(name="psU", bufs=4, space="PSUM") as psUp,
        ):
            nc.gpsimd.load_library(library_config.mlp)

            bmall = mskp.tile([P, n_bm_cols], F16)
            nc.sync.dma_start(bmall[:], BM[:])
            small = mskp.tile([P, P * nkeys], F16)
            nc.sync.dma_start(small[:], SM[:])
            bmtall = mskp.tile([32, P * nkeys], F16)
            nc.sync.dma_start(bmtall[:], BMT[:])
            i128 = mskp.tile([P, P], F16)
            nc.sync.dma_start(i128[:], I128[:])
            w2r = mskp.tile([P, 2 * HIDDEN], F16)
            nc.sync.dma_start(w2r[:], W2R[:])
            jtall = mskp.tile([P, total_idx // 16], mybir.dt.int16)
            nc.sync.dma_start(jtall[:], IDX[:])

            i0 = 0
            co = 0
            call = 0
            for (s, S, n_tiles, m) in program:
                ki = key_idx[(S, m)]
                t = 0
                while t < n_tiles:
                    nb = min(NBT, n_tiles - t)
                    nidx = nb * P
                    G = gp.tile([P, NBT, ROW], F16, tag="G")
                    nc.gpsimd.dma_gather(
                        out_ap=G[:, :nb, :],
                        in_ap=XP[int(seg_base[s]):int(seg_base[s + 1]), :],
                        idxs_ap=jtall[:, i0 // 16:(i0 + nidx) // 16],
                        num_idxs=nidx,
                        num_idxs_reg=nidx,
                        elem_size=ROW,
                        single_packet=False,
                        queue_num=call % NQUEUES,
                    )
                    call += 1
                    # per-slot scores: sj, si via mul + free-dim reduce
                    tmp = tmp_p.tile([P, NBT, 2, HIDDEN], F16, tag="tmp")
                    nc.vector.tensor_mul(
                        tmp[:, :nb, :, :],
                        G[:, :nb, 0:HIDDEN].unsqueeze(2).broadcast_to(
                            [P, nb, 2, HIDDEN]),
                        w2r[:].rearrange("p (w e) -> p w e", w=2)
                        .unsqueeze(1).broadcast_to([P, nb, 2, HIDDEN]))
                    sco = scp.tile([P, NBT, 2], F16, tag="sco")
                    with nc.allow_low_precision(reason="fp16 scores |s|<8"):
                        nc.vector.tensor_reduce(
                            sco[:, :nb, :], tmp[:, :nb, :, :],
                            axis=mybir.AxisListType.X, op=mybir.AluOpType.add)
                    # raw score: psc0[p,k] = si[slot0(p)] + sj[p]
                    psc = pscp.tile([P, 3, NBT], mybir.dt.float32, tag="psc")
                    nc.tensor.matmul(
                        psc[:, 0, :nb],
                        lhsT=small[:, ki * P:(ki + 1) * P],
                        rhs=sco[:, :nb, 1],
                        start=True, stop=False)
                    nc.tensor.matmul(
                        psc[:, 0, :nb],
                        lhsT=i128[:],
                        rhs=sco[:, :nb, 0],
                        start=False, stop=True)
                    # leaky relu on DVE (keeps Act at Exp/Relu only)
                    esc = scp.tile([P, NBT], mybir.dt.float32, tag="esc")
                    nc.vector.tensor_scalar_mul(esc[:, :nb], psc[:, 0, :nb],
                                                LEAKY)
                    elr = scp.tile([P, NBT], F16, tag="elr")
                    nc.vector.tensor_max(elr[:, :nb], psc[:, 0, :nb],
                                         esc[:, :nb])
                    ex = scp.tile([P, NBT], F16, tag="ex")
                    nc.scalar.activation(ex[:, :nb], elr[:, :nb], AFT.Exp)
                    exm = scp.tile([P, NBT], F16, tag="exm")
                    nc.vector.tensor_mul(exm[:, :nb], ex[:, :nb],
                                         G[:, :nb, HIDDEN])
                    # denominators per block -> per-slot reciprocal
                    nc.tensor.matmul(
                        psc[0:m, 1, :nb],
                        lhsT=bmall[:, int(bm_off[ki]):int(bm_off[ki]) + m],
                        rhs=exm[:, :nb],
                        start=True, stop=True)
                    rec = scp.tile([32, NBT], F16, tag="rec")
                    with nc.allow_low_precision(
                            reason="fp16 1/denom; denom in [0.9, 9e4]"):
                        nc.vector.reciprocal(rec[:m, :nb],
                                             psc[0:m, 1, :nb])
                    nc.tensor.matmul(
                        psc[:, 2, :nb],
                        lhsT=bmtall[0:m, ki * P:(ki + 1) * P],
                        rhs=rec[:m, :nb],
                        start=True, stop=True)
                    exr = scp.tile([P, NBT], F16, tag="exr")
                    nc.vector.tensor_mul(exr[:, :nb], exm[:, :nb],
                                         psc[:, 2, :nb])
                    exsel = esp.tile([P, NBT, m], F16, tag="exsel")
                    nc.vector.tensor_mul(
                        exsel[:, :nb, :],
                        bmall[:, int(bm_off[ki]):int(bm_off[ki]) + m
                              ].unsqueeze(1).broadcast_to([P, nb, m]),
                        exr[:, :nb].unsqueeze(2).broadcast_to([P, nb, m]))
                    # aggregation groups
                    ngc = (nb + GMAX - 1) // GMAX
                    stage = stp.tile([P, GMAX, GMAX, HIDDEN], F16,
                                     tag="stage")
                    gslot = 0
                    k0 = 0
                    while k0 < nb:
                        g = min(GMAX, nb - k0)
                        psU = psUp.tile([P, GMAX, HIDDEN], mybir.dt.float32,
                                        tag="psU")
                        nc.tensor.matmul(
                            psU[0:g * m, 0:g, :],
                            lhsT=exsel[:, k0:k0 + g, :],
                            rhs=G[:, k0:k0 + g, 0:HIDDEN],
                            start=True, stop=True)
                        nc.scalar.activation(
                            stage[0:g * m, gslot, 0:g, :],
                            psU[0:g * m, 0:g, :], AFT.Relu)
                        gslot += 1
                        k0 += g
                    # row = co + p*ngc + gslot: contiguous per partition
                    nc.sync.dma_start(
                        OUT[co:co + P * ngc, :].rearrange(
                            "(p g) e -> p g e", g=ngc),
                        stage[:, :ngc, :, :])
                    co += P * ngc
                    i0 += nidx
                    t += nb
    nc.compile()
    return nc


def _install_profhook():
    """Register the axon NTFF profile hook (missing glue in this container)."""
    import contextlib
    import ctypes
    import sys
    import types

    if "antenv.axon_hooks" in sys.modules:
        return
    try:
        lib = ctypes.CDLL("/opt/axon/libaxon_pjrt.so")
        assert hasattr(lib, "axon_start_nrt_profile")
    except Exception:
        return
    lib.axon_start_nrt_profile.argtypes = [ctypes.POINTER(ctypes.c_int64),
                                           ctypes.c_size_t]
    lib.axon_start_nrt_profile.restype = ctypes.c_int64
    lib.axon_stop_nrt_profile.argtypes = [ctypes.c_char_p]
    lib.axon_stop_nrt_profile.restype = ctypes.c_int64

    @contextlib.contextmanager
    def _hook(output_dir, device_ids):
        import jax

        jax.devices()
        if device_ids:
            ids = (ctypes.c_int64 * len(device_ids))(*device_ids)
            rc = lib.axon_start_nrt_profile(ids, len(device_ids))
        else:
            rc = lib.axon_start_nrt_profile(None, 0)
        if rc != 0:
            raise RuntimeError(f"axon_start_nrt_profile rc={rc}")
        try:
            yield
        finally:
            lib.axon_stop_nrt_profile(str(output_dir).encode())

    mod = types.ModuleType("antenv.axon_hooks")
    mod.get_axon_ntff_profile_hook = lambda: _hook
    mod.set_axon_ntff_profile_hook = lambda h: None
    sys.modules["antenv.axon_hooks"] = mod
    import antenv

    antenv.axon_hooks = mod


def kernel(x, edge_index, w_i, w_j):
    import os
    from concourse.bass_utils import run_bass_kernel_spmd

    f16 = np.float16
    x = np.asarray(x, dtype=np.float32)
    edge_index = np.asarray(edge_index)
    w_i = np.asarray(w_i, dtype=np.float32)
    w_j = np.asarray(w_j, dtype=np.float32)
    n = x.shape[0]
    assert n == N_NODES and x.shape[1] == HIDDEN
    npc = n // N_CORES

    ej = edge_index[0].astype(np.int64)
    ei = edge_index[1].astype(np.int64)
    core_of = ei // npc
    edge_src, edge_dstl = [], []
    for c in range(N_CORES):
        sel = core_of == c
        edge_src.append(ej[sel])
        edge_dstl.append(ei[sel] - c * npc)

    (program, total_tiles, total_out_rows, Js,
     grows, gcols, gnodes, seg_nodes) = \
        _build_layout(edge_src, edge_dstl, npc)
    bmc, smc, bmtc, bm_off, key_idx = _build_masks(program)

    # per-segment table sizes uniform across cores (one shared program);
    # +2 for the sentinel rows
    seg_rows = [2 + max(len(seg_nodes[c][s]) for c in range(N_CORES))
                for s in range(NSEG)]

    W2r = np.tile(np.concatenate([w_j, w_i]).astype(f16)[None, :], (P, 1))
    I128c = np.eye(P, dtype=f16)

    nc = _build_program(program, total_tiles, total_out_rows, seg_rows,
                        bmc.shape[1], len(key_idx))

    x16 = x.astype(f16)
    in_maps = []
    for c in range(N_CORES):
        tabs = []
        for s in range(NSEG):
            nodes = seg_nodes[c][s]
            t = np.zeros((seg_rows[s], ROW), dtype=f16)
            t[1, HIDDEN] = 1.0  # flag-1 sentinel (zero x)
            t[2:2 + len(nodes), 0:HIDDEN] = x16[nodes]
            t[2:2 + len(nodes), HIDDEN] = 1.0
            tabs.append(t)
        XPc = np.ascontiguousarray(np.concatenate(tabs, 0))
        idxmat = np.ascontiguousarray(Js[c].reshape(-1, 16).T)
        IDXc = np.ascontiguousarray(np.tile(idxmat, (8, 1)))
        in_maps.append({
            "XP": XPc, "IDX": IDXc,
            "W2R": np.ascontiguousarray(W2r),
            "BM": np.ascontiguousarray(bmc),
            "SM": np.ascontiguousarray(smc),
            "BMT": np.ascontiguousarray(bmtc),
            "I128": np.ascontiguousarray(I128c),
        })
    trace = os.environ.get("GAT_TRACE") == "1"
    if trace:
        _install_profhook()
    res = run_bass_kernel_spmd(nc, in_maps, core_ids=list(range(N_CORES)),
                               trace=trace)
    if trace and res.exec_time_ns:
        print(f"HW exec time: {res.exec_time_ns} ns")

    out = np.zeros((n, HIDDEN), dtype=np.float32)
    for c in range(N_CORES):
        ot = res.results[c]["OUT"].reshape(total_out_rows, GMAX, HIDDEN)
        out[c * npc + gnodes[c]] = ot[grows[c], gcols[c], :].astype(
            np.float32)
    return out


# revision 15
# speedup vs baseline: 2.4424x; 1.3798x over previous
"""GAT message-passing kernel for 8 TRN2 NeuronCores (Bass/Tile).

v5.1 strategy (dst-sharded, PE/Act-heavy softmax, no collectives):
  - Each core owns a contiguous range of destination nodes; the host routes
    each edge to the core owning its destination (edge_index[1]).  8 dst
    segments per core keep compact gather tables within int16 index space
    (last segment takes the remainder — v4 dropped the last 4 dsts/core).
  - Edges grouped per destination into blocks of S slots (slot 0 = the
    destination's own row, slots 1..deg = sources, rest point at the
    all-zero sentinel row 0).  Degrees pool into size classes so buckets
    stay large: less cross-core tile padding, fewer (bucket-bound) chunks.
    Table row fp16 [x(64) | flag | pad]; flag=0 on the sentinel masks the
    padded slots out of the softmax.
  - Per chunk of up to 16 tiles (128 slots each):
      dma_gather (4 SWDGE queues rotating, ~2.3ns/row) -> G [128, nb, 128]
      DVE: tmp = G.x * [wj|wi]; sco = reduce -> per-slot sj, si (fp16)
      PE:  psc = SM @ si + I128 @ sj        raw score (slot0 bcast + own sj)
      DVE: lrelu (mul + max);  Act: Exp -> ex fp16   (Act runs ONLY
           Exp/Relu — mixing in Lrelu/Copy thrashed the act table cache)
      DVE: exm = ex * flag;  PE: den = BM-contract(exm);  DVE: rec = 1/den
      PE:  recsel = BMT @ rec -> per-slot 1/denominator
      DVE: exr = exm * recsel; exsel = BM x exr      normalized alpha
      PE:  per <=4-tile group one matmul, stationary exsel [128, g*m],
           moving G.x [128, g, 64] -> psU [g*m, g, 64] (g-1 junk bands —
           junk costs no extra PE columns and the host skips it)
      Act: Relu psU -> fp16 stage
      one DMA per chunk -> OUT, row = base + p*ngc + group: each partition
      writes one contiguous run (the v5 (g p) order scattered 512B bursts)
  - Host extracts the diagonal bands from OUT and assembles the result.
"""
import numpy as np

N_NODES = 100000
HIDDEN = 64
N_CORES = 8
NSEG = 8                 # dst segments per core (int16 index headroom)
LEAKY = 0.01
P = 128
ROW = 128                # fp16 elements per table row (256B, dma_gather)
NBT = 16                 # tiles per gather chunk
GMAX = 4                 # tiles per aggregation group (4m <= 128)
NQUEUES = 4

# block slot-count classes (S = degree+1 rounded up to one of these)
SCLASSES = [2, 3, 4, 5, 6, 7, 8, 9, 10, 11, 12, 14, 16, 18, 20, 23, 26,
            29, 33, 37, 42, 47, 53, 60, 67, 75, 84, 94, 106, 118, 127]


def _sclass(d):
    S = d + 1
    for sc in SCLASSES:
        if sc >= S:
            return sc
    raise ValueError(f"unsupported degree {d}")


def _build_layout(edge_src, edge_dst_local, nodes_per_core):
    """Per-core, per-segment compact tables + class-bucket/chunk structure."""
    ncores = len(edge_src)
    npseg = -(-nodes_per_core // NSEG)  # ceil: last segment holds remainder

    # per (core, seg): {Sclass: [(local dst, srcs), ...]}
    seg_info = []
    for c in range(ncores):
        src, dstl = edge_src[c], edge_dst_local[c]
        order = np.argsort(dstl, kind="stable")
        src, dstl = src[order], dstl[order]
        deg = np.bincount(dstl, minlength=nodes_per_core)
        starts = np.concatenate([[0], np.cumsum(deg)])
        per_seg = []
        for s in range(NSEG):
            lo = s * npseg
            hi = min((s + 1) * npseg, nodes_per_core)
            per = {}
            for n in range(lo, hi):
                d = int(deg[n])
                if d == 0:
                    continue
                per.setdefault(_sclass(d), []).append(
                    (n, src[starts[n]:starts[n + 1]]))
            per_seg.append(per)
        seg_info.append(per_seg)

    # shared bucket structure: n_tiles = max over cores
    program = []  # (seg, S, n_tiles, m)
    for s in range(NSEG):
        all_S = sorted({S for c in range(ncores)
                        for S in seg_info[c][s].keys()})
        for S in all_S:
            m = min(P // S, 32)
            maxb = max(len(seg_info[c][s].get(S, [])) for c in range(ncores))
            n_tiles = (maxb + m - 1) // m
            program.append((s, S, n_tiles, m))

    total_tiles = sum(p[2] for p in program)
    total_idx = total_tiles * P

    # chunk walk (shared): chunk list of (s, S, m, nb, ngc); groups g<=GMAX
    chunks = []
    for (s, S, n_tiles, m) in program:
        t = 0
        while t < n_tiles:
            nb = min(NBT, n_tiles - t)
            ngc = (nb + GMAX - 1) // GMAX
            chunks.append((s, S, m, nb, ngc))
            t += nb
    total_out_rows = sum(P * ch[4] for ch in chunks)

    # per-core: index streams + output extraction maps
    Js = []
    grows, gcols, gnodes = [], [], []
    seg_nodes = []  # [core][seg] -> global node ids (row r+1 = nodes[r])
    for c in range(ncores):
        J = np.zeros(total_idx, dtype=np.int16)
        er, ec, en = [], [], []
        pernodes = []
        i0 = 0
        co = 0  # OUT row base of current chunk
        base_global = c * nodes_per_core
        for s in range(NSEG):
            srcs_all = [srcs for S, lst in seg_info[c][s].items()
                        for (_, srcs) in lst]
            lo = s * npseg
            hi = min((s + 1) * npseg, nodes_per_core)
            dsts_all = np.arange(lo, hi) + base_global
            allref = np.concatenate(
                [np.concatenate(srcs_all) if srcs_all else
                 np.empty(0, dtype=np.int64), dsts_all])
            nodes = np.unique(allref)
            assert len(nodes) <= 32765, len(nodes)
            pernodes.append(nodes)
            # row0 = flag-0 sentinel (class pads), row1 = flag-1 sentinel
            # (pad blocks: den=1, avoids inf/NaN), real rows from 2
            lut = {int(n): j + 2 for j, n in enumerate(nodes)}
            for (s2, S, n_tiles, m) in program:
                if s2 != s:
                    continue
                lst = seg_info[c][s].get(S, [])
                for bi, (n, srcs) in enumerate(lst):
                    t, b = bi // m, bi % m
                    base = i0 + t * P + b * S
                    J[base] = lut[int(n) + base_global]
                    for e, sv in enumerate(srcs):
                        J[base + 1 + e] = lut[int(sv)]
                nblocks = len(lst)
                for bi in range(nblocks, n_tiles * m):
                    t2, b2 = bi // m, bi % m
                    J[i0 + t2 * P + b2 * S + 1] = 1
                t = 0
                while t < n_tiles:
                    nb = min(NBT, n_tiles - t)
                    ngc = (nb + GMAX - 1) // GMAX
                    k0 = 0
                    gslot = 0
                    while k0 < nb:
                        g = min(GMAX, nb - k0)
                        for k in range(g):
                            tabs = t + k0 + k
                            for b in range(m):
                                bi = tabs * m + b
                                if bi < nblocks:
                                    er.append(co + (k * m + b) * ngc + gslot)
                                    ec.append(k)
                                    en.append(lst[bi][0])
                        gslot += 1
                        k0 += g
                    co += P * ngc
                    t += nb
                i0 += n_tiles * P
        assert co == total_out_rows
        Js.append(J)
        grows.append(np.array(er, dtype=np.int64))
        gcols.append(np.array(ec, dtype=np.int64))
        gnodes.append(np.array(en, dtype=np.int64))
        seg_nodes.append(pernodes)
    return (program, total_tiles, total_out_rows, Js,
            grows, gcols, gnodes, seg_nodes)


def _build_masks(program):
    """Per-bucket masks: BM [P, m] (src slots), SM [P, P] (slot0 select),
    BMT [32, P] (BM transpose)."""
    keys = sorted({(S, m) for (_, S, _, m) in program})
    bm, sm, bmt, key_idx = [], [], [], {}
    for ki, (S, m) in enumerate(keys):
        B = np.zeros((P, m), dtype=np.float32)
        SEL = np.zeros((P, P), dtype=np.float32)
        BT = np.zeros((32, P), dtype=np.float32)
        for p in range(m * S):
            if p % S != 0:
                B[p, p // S] = 1.0
                BT[p // S, p] = 1.0
            SEL[(p // S) * S, p] = 1.0
        bm.append(B)
        sm.append(SEL)
        bmt.append(BT)
        key_idx[(S, m)] = ki
    bmc = np.concatenate(bm, 1).astype(np.float16)
    smc = np.concatenate(sm, 1).astype(np.float16)
    bmtc = np.concatenate(bmt, 1).astype(np.float16)
    bm_off = np.cumsum([0] + [b.shape[1] for b in bm])
    return bmc, smc, bmtc, bm_off, key_idx


def _build_program(program, total_tiles, total_out_rows, seg_rows, n_bm_cols,
                   nkeys):
    import concourse.bass as bass  # noqa: F401
    import concourse.tile as tile
    from concourse import bacc, mybir, library_config
    from concourse.mybir import ActivationFunctionType as AFT

    total_idx = total_tiles * P
    seg_base = np.cumsum([0] + list(seg_rows))
    F16 = mybir.dt.float16

    nc = bacc.Bacc("TRN2", target_bir_lowering=False,
                   num_swdge_queues=NQUEUES,
                   dynamic_dma_scratch_size=65536)
    XP = nc.dram_tensor("XP", [int(seg_base[-1]), ROW], F16,
                        kind="ExternalInput")
    IDX = nc.dram_tensor("IDX", [P, total_idx // 16], mybir.dt.int16,
                         kind="ExternalInput")
    W2R = nc.dram_tensor("W2R", [P, 2 * HIDDEN], F16, kind="ExternalInput")
    BM = nc.dram_tensor("BM", [P, n_bm_cols], F16, kind="ExternalInput")
    SM = nc.dram_tensor("SM", [P, P * nkeys], F16, kind="ExternalInput")
    BMT = nc.dram_tensor("BMT", [32, P * nkeys], F16, kind="ExternalInput")
    I128 = nc.dram_tensor("I128", [P, P], F16, kind="ExternalInput")
    OUT = nc.dram_tensor("OUT", [total_out_rows, GMAX * HIDDEN], F16,
                         kind="ExternalOutput")

    _, _, _, bm_off, key_idx = _build_masks(program)

    with tile.TileContext(nc) as tc:
        with (
            tc.tile_pool(name="msk", bufs=1) as mskp,
            tc.tile_pool(name="g", bufs=8) as gp,
            tc.tile_pool(name="tm", bufs=4) as tmp_p,
            tc.tile_pool(name="sc", bufs=6) as scp,
            tc.tile_pool(name="es", bufs=4) as esp,
            tc.tile_pool(name="st", bufs=4) as stp,
            tc.tile_pool(name="psc", bufs=4, space="PSUM") as pscp,
            tc.tile_pool(name="psU", bufs=4, space="PSUM") as psUp,
        ):
            nc.gpsimd.load_library(library_config.mlp)

            bmall = mskp.tile([P, n_bm_cols], F16)
            nc.sync.dma_start(bmall[:], BM[:])
            small = mskp.tile([P, P * nkeys], F16)
            nc.sync.dma_start(small[:], SM[:])
            bmtall = mskp.tile([32, P * nkeys], F16)
            nc.sync.dma_start(bmtall[:], BMT[:])
            i128 = mskp.tile([P, P], F16)
            nc.sync.dma_start(i128[:], I128[:])
            w2r = mskp.tile([P, 2 * HIDDEN], F16)
            nc.sync.dma_start(w2r[:], W2R[:])
            jtall = mskp.tile([P, total_idx // 16], mybir.dt.int16)
            nc.sync.dma_start(jtall[:], IDX[:])

            i0 = 0
            co = 0
            call = 0
            for (s, S, n_tiles, m) in program:
                ki = key_idx[(S, m)]
                t = 0
                while t < n_tiles:
                    nb = min(NBT, n_tiles - t)
                    nidx = nb * P
                    G = gp.tile([P, NBT, ROW], F16, tag="G")
                    nc.gpsimd.dma_gather(
                        out_ap=G[:, :nb, :],
                        in_ap=XP[int(seg_base[s]):int(seg_base[s + 1]), :],
                        idxs_ap=jtall[:, i0 // 16:(i0 + nidx) // 16],
                        num_idxs=nidx,
                        num_idxs_reg=nidx,
                        elem_size=ROW,
                        single_packet=False,
                        queue_num=call % NQUEUES,
                    )
                    call += 1
                    # per-slot scores: sj, si via mul + free-dim reduce
                    tmp = tmp_p.tile([P, NBT, 2, HIDDEN], F16, tag="tmp")
                    nc.vector.tensor_mul(
                        tmp[:, :nb, :, :],
                        G[:, :nb, 0:HIDDEN].unsqueeze(2).broadcast_to(
                            [P, nb, 2, HIDDEN]),
                        w2r[:].rearrange("p (w e) -> p w e", w=2)
                        .unsqueeze(1).broadcast_to([P, nb, 2, HIDDEN]))
                    sco = scp.tile([P, NBT, 2], F16, tag="sco")
                    with nc.allow_low_precision(reason="fp16 scores |s|<8"):
                        nc.vector.tensor_reduce(
                            sco[:, :nb, :], tmp[:, :nb, :, :],
                            axis=mybir.AxisListType.X, op=mybir.AluOpType.add)
                    # raw score: psc0[p,k] = si[slot0(p)] + sj[p]
                    psc = pscp.tile([P, 3, NBT], mybir.dt.float32, tag="psc")
                    nc.tensor.matmul(
                        psc[:, 0, :nb],
                        lhsT=small[:, ki * P:(ki + 1) * P],
                        rhs=sco[:, :nb, 1],
                        start=True, stop=False)
                    nc.tensor.matmul(
                        psc[:, 0, :nb],
                        lhsT=i128[:],
                        rhs=sco[:, :nb, 0],
                        start=False, stop=True)
                    # leaky relu on DVE (keeps Act at Exp/Relu only)
                    esc = scp.tile([P, NBT], mybir.dt.float32, tag="esc")
                    nc.vector.tensor_scalar_mul(esc[:, :nb], psc[:, 0, :nb],
                                                LEAKY)
                    elr = scp.tile([P, NBT], F16, tag="elr")
                    nc.vector.tensor_max(elr[:, :nb], psc[:, 0, :nb],
                                         esc[:, :nb])
                    ex = scp.tile([P, NBT], F16, tag="ex")
                    nc.scalar.activation(ex[:, :nb], elr[:, :nb], AFT.Exp)
                    exm = scp.tile([P, NBT], F16, tag="exm")
                    nc.vector.tensor_mul(exm[:, :nb], ex[:, :nb],
                                         G[:, :nb, HIDDEN])
                    # denominators per block -> per-slot reciprocal
                    nc.tensor.matmul(
                        psc[0:m, 1, :nb],
                        lhsT=bmall[:, int(bm_off[ki]):int(bm_off[ki]) + m],
                        rhs=exm[:, :nb],
                        start=True, stop=True)
                    rec = scp.tile([32, NBT], F16, tag="rec")
                    with nc.allow_low_precision(
                            reason="fp16 1/denom; denom in [0.9, 9e4]"):
                        nc.vector.reciprocal(rec[:m, :nb],
                                             psc[0:m, 1, :nb])
                    nc.tensor.matmul(
                        psc[:, 2, :nb],
                        lhsT=bmtall[0:m, ki * P:(ki + 1) * P],
                        rhs=rec[:m, :nb],
                        start=True, stop=True)
                    exr = scp.tile([P, NBT], F16, tag="exr")
                    nc.vector.tensor_mul(exr[:, :nb], exm[:, :nb],
                                         psc[:, 2, :nb])
                    exsel = esp.tile([P, NBT, m], F16, tag="exsel")
                    nc.vector.tensor_mul(
                        exsel[:, :nb, :],
                        bmall[:, int(bm_off[ki]):int(bm_off[ki]) + m
                              ].unsqueeze(1).broadcast_to([P, nb, m]),
                        exr[:, :nb].unsqueeze(2).broadcast_to([P, nb, m]))
                    # aggregation groups
                    ngc = (nb + GMAX - 1) // GMAX
                    stage = stp.tile([P, GMAX, GMAX, HIDDEN], F16,
                                     tag="stage")
                    gslot = 0
                    k0 = 0
                    while k0 < nb:
                        g = min(GMAX, nb - k0)
                        psU = psUp.tile([P, GMAX, HIDDEN], mybir.dt.float32,
                                        tag="psU")
                        nc.tensor.matmul(
                            psU[0:g * m, 0:g, :],
                            lhsT=exsel[:, k0:k0 + g, :],
                            rhs=G[:, k0:k0 + g, 0:HIDDEN],
                            start=True, stop=True)
                        nc.scalar.activation(
                            stage[0:g * m, gslot, 0:g, :],
                            psU[0:g * m, 0:g, :], AFT.Relu)
                        gslot += 1
                        k0 += g
                    # row = co + p*ngc + gslot: contiguous per partition
                    nc.sync.dma_start(
                        OUT[co:co + P * ngc, :].rearrange(
                            "(p g) e -> p g e", g=ngc),
                        stage[:, :ngc, :, :])
                    co += P * ngc
                    i0 += nidx
                    t += nb
    nc.compile()
    return nc


def _install_profhook():
    """Register the axon NTFF profile hook (missing glue in this container)."""
    import contextlib
    import ctypes
    import sys
    import types

    if "antenv.axon_hooks" in sys.modules:
        return
    try:
        lib = ctypes.CDLL("/opt/axon/libaxon_pjrt.so")
        assert hasattr(lib, "axon_start_nrt_profile")
    except Exception:
        return
    lib.axon_start_nrt_profile.argtypes = [ctypes.POINTER(ctypes.c_int64),
                                           ctypes.c_size_t]
    lib.axon_start_nrt_profile.restype = ctypes.c_int64
    lib.axon_stop_nrt_profile.argtypes = [ctypes.c_char_p]
    lib.axon_stop_nrt_profile.restype = ctypes.c_int64

    @contextlib.contextmanager
    def _hook(output_dir, device_ids):
        import jax

        jax.devices()
        if device_ids:
            ids = (ctypes.c_int64 * len(device_ids))(*device_ids)
            rc = lib.axon_start_nrt_profile(ids, len(device_ids))
        else:
            rc = lib.axon_start_nrt_profile(None, 0)
        if rc != 0:
            raise RuntimeError(f"axon_start_nrt_profile rc={rc}")
        try:
            yield
        finally:
            lib.axon_stop_nrt_profile(str(output_dir).encode())

    mod = types.ModuleType("antenv.axon_hooks")
    mod.get_axon_ntff_profile_hook = lambda: _hook
    mod.set_axon_ntff_profile_hook = lambda h: None
    sys.modules["antenv.axon_hooks"] = mod
    import antenv

    antenv.axon_hooks = mod


def kernel(x, edge_index, w_i, w_j):
    import os
    from concourse.bass_utils import run_bass_kernel_spmd

    f16 = np.float16
    x = np.asarray(x, dtype=np.float32)
    edge_index = np.asarray(edge_index)
    w_i = np.asarray(w_i, dtype=np.float32)
    w_j = np.asarray(w_j, dtype=np.float32)
    n = x.shape[0]
    assert n == N_NODES and x.shape[1] == HIDDEN
    npc = n // N_CORES

    ej = edge_index[0].astype(np.int64)
    ei = edge_index[1].astype(np.int64)
    core_of = ei // npc
    edge_src, edge_dstl = [], []
    for c in range(N_CORES):
        sel = core_of == c
        edge_src.append(ej[sel])
        edge_dstl.append(ei[sel] - c * npc)

    (program, total_tiles, total_out_rows, Js,
     grows, gcols, gnodes, seg_nodes) = \
        _build_layout(edge_src, edge_dstl, npc)
    bmc, smc, bmtc, bm_off, key_idx = _build_masks(program)

    # per-segment table sizes uniform across cores (one shared program);
    # +2 for the sentinel rows
    seg_rows = [2 + max(len(seg_nodes[c][s]) for c in range(N_CORES))
                for s in range(NSEG)]

    W2r = np.tile(np.concatenate([w_j, w_i]).astype(f16)[None, :], (P, 1))
    I128c = np.eye(P, dtype=f16)

    nc = _build_program(program, total_tiles, total_out_rows, seg_rows,
                        bmc.shape[1], len(key_idx))

    x16 = x.astype(f16)
    in_maps = []
    for c in range(N_CORES):
        tabs = []
        for s in range(NSEG):
            nodes = seg_nodes[c][s]
            t = np.zeros((seg_rows[s], ROW), dtype=f16)
            t[1, HIDDEN] = 1.0  # flag-1 sentinel (zero x)
            t[2:2 + len(nodes), 0:HIDDEN] = x16[nodes]
            t[2:2 + len(nodes), HIDDEN] = 1.0
            tabs.append(t)
        XPc = np.ascontiguousarray(np.concatenate(tabs, 0))
        idxmat = np.ascontiguousarray(Js[c].reshape(-1, 16).T)
        IDXc = np.ascontiguousarray(np.tile(idxmat, (8, 1)))
        in_maps.append({
            "XP": XPc, "IDX": IDXc,
            "W2R": np.ascontiguousarray(W2r),
            "BM": np.ascontiguousarray(bmc),
            "SM": np.ascontiguousarray(smc),
            "BMT": np.ascontiguousarray(bmtc),
            "I128": np.ascontiguousarray(I128c),
        })
    trace = os.environ.get("GAT_TRACE") == "1"
    if trace:
        _install_profhook()
    res = run_bass_kernel_spmd(nc, in_maps, core_ids=list(range(N_CORES)),
                               trace=trace)
    if trace and res.exec_time_ns:
        print(f"HW exec time: {res.exec_time_ns} ns")

    out = np.zeros((n, HIDDEN), dtype=np.float32)
    for c in range(N_CORES):
        ot = res.results[c]["OUT"].reshape(total_out_rows, GMAX, HIDDEN)
        out[c * npc + gnodes[c]] = ot[grows[c], gcols[c], :].astype(
            np.float32)
    return out


# revision 23
# speedup vs baseline: 4.4378x; 1.8170x over previous
"""GAT message-passing kernel for 8 TRN2 NeuronCores (Bass/Tile).

v5.1 strategy (dst-sharded, PE/Act-heavy softmax, no collectives):
  - Each core owns a contiguous range of destination nodes; the host routes
    each edge to the core owning its destination (edge_index[1]).  8 dst
    segments per core keep compact gather tables within int16 index space
    (last segment takes the remainder — v4 dropped the last 4 dsts/core).
  - Edges grouped per destination into blocks of S slots (slot 0 = the
    destination's own row, slots 1..deg = sources, rest point at the
    all-zero sentinel row 0).  Degrees pool into size classes so buckets
    stay large: less cross-core tile padding, fewer (bucket-bound) chunks.
    Table row fp16 [x(64) | flag | pad]; flag=0 on the sentinel masks the
    padded slots out of the softmax.
  - Per chunk of up to 16 tiles (128 slots each):
      dma_gather (4 SWDGE queues rotating, ~2.3ns/row) -> G [128, nb, 128]
      DVE: tmp = G.x * [wj|wi]; sco = reduce -> per-slot sj, si (fp16)
      PE:  psc = SM @ si + I128 @ sj        raw score (slot0 bcast + own sj)
      DVE: lrelu (mul + max);  Act: Exp -> ex fp16   (Act runs ONLY
           Exp/Relu — mixing in Lrelu/Copy thrashed the act table cache)
      DVE: exm = ex * flag;  PE: den = BM-contract(exm);  DVE: rec = 1/den
      PE:  recsel = BMT @ rec -> per-slot 1/denominator
      DVE: exr = exm * recsel; exsel = BM x exr      normalized alpha
      PE:  per <=4-tile group one matmul, stationary exsel [128, g*m],
           moving G.x [128, g, 64] -> psU [g*m, g, 64] (g-1 junk bands —
           junk costs no extra PE columns and the host skips it)
      Act: Relu psU -> fp16 stage
      one DMA per chunk -> OUT, row = base + p*ngc + group: each partition
      writes one contiguous run (the v5 (g p) order scattered 512B bursts)
  - Host extracts the diagonal bands from OUT and assembles the result.
"""
import numpy as np

N_NODES = 100000
HIDDEN = 64
N_CORES = 8
NSEG = 8                 # dst segments per core (int16 index headroom)
LEAKY = 0.01
P = 128
ROW = 128                # fp16 elements per table row (256B, dma_gather)
NBT = 16                 # tiles per gather chunk
GMAX = 4                 # tiles per aggregation group (4m <= 128)
NQUEUES = 4
NS0 = 256                # flag-0 sentinel rows (spread: avoids DRAM hotspot)
NS1 = 32                 # flag-1 sentinel rows (pad blocks, den=1)

# block slot-count classes (S = degree+1 rounded up to one of these)
SCLASSES = [2, 3, 4, 5, 6, 7, 8, 9, 10, 11, 12, 14, 16, 18, 20, 23, 26,
            29, 33, 37, 42, 47, 53, 60, 67, 75, 84, 94, 106, 118, 127]


def _sclass(d):
    S = d + 1
    for sc in SCLASSES:
        if sc >= S:
            return sc
    raise ValueError(f"unsupported degree {d}")


def _build_layout(ej, ei, n_nodes, ncores):
    """Per-core, per-segment compact tables + class-bucket/chunk structure.

    Destinations are dealt to (core, segment) round-robin by degree rank so
    every (core, seg, class) bucket has a near-identical block count — the
    shared program pads each bucket to the max over cores, so contiguous
    range sharding wasted ~18% of the gather rows on padding.
    """
    deg = np.bincount(ei, minlength=n_nodes)
    order = np.argsort(ei, kind="stable")
    src_sorted = ej[order]
    starts = np.concatenate([[0], np.cumsum(deg)])
    nz = np.where(deg > 0)[0]
    rank = nz[np.argsort(-deg[nz], kind="stable")]
    # rank r -> core r%ncores, segment (r//ncores)%NSEG
    seg_info = [[{} for _ in range(NSEG)] for _ in range(ncores)]
    for r, n in enumerate(rank):
        c = r % ncores
        s = (r // ncores) % NSEG
        d = int(deg[n])
        seg_info[c][s].setdefault(_sclass(d), []).append(
            (int(n), src_sorted[starts[n]:starts[n + 1]]))

    # shared bucket structure: n_tiles = max over cores
    program = []  # (seg, S, n_tiles, m)
    for s in range(NSEG):
        all_S = sorted({S for c in range(ncores)
                        for S in seg_info[c][s].keys()})
        for S in all_S:
            m = min(P // S, 32)
            maxb = max(len(seg_info[c][s].get(S, [])) for c in range(ncores))
            n_tiles = (maxb + m - 1) // m
            program.append((s, S, n_tiles, m))

    total_tiles = sum(p[2] for p in program)
    total_idx = total_tiles * P

    # chunk walk (shared): chunk list of (s, S, m, nb, ngc); groups g<=GMAX
    chunks = []
    for (s, S, n_tiles, m) in program:
        t = 0
        while t < n_tiles:
            nb = min(NBT, n_tiles - t)
            ngc = (nb + GMAX - 1) // GMAX
            chunks.append((s, S, m, nb, ngc))
            t += nb
    total_out_rows = sum(P * ch[4] for ch in chunks)

    # per-core: index streams + output extraction maps
    rng = np.random.default_rng(1234)
    NS = NS0 + NS1
    Js = []
    grows, gcols, gnodes = [], [], []
    seg_nodes = []  # [core][seg] -> global node ids (row NS+r = nodes[r])
    for c in range(ncores):
        J = rng.integers(0, NS0, size=total_idx).astype(np.int16)
        er, ec, en = [], [], []
        pernodes = []
        i0 = 0
        co = 0  # OUT row base of current chunk
        for s in range(NSEG):
            allref = [srcs for S, lst in seg_info[c][s].items()
                      for (_, srcs) in lst]
            allref.append(np.array(
                [n for S, lst in seg_info[c][s].items()
                 for (n, _) in lst], dtype=np.int64))
            nodes = np.unique(np.concatenate(allref)) if allref else \
                np.empty(0, dtype=np.int64)
            assert len(nodes) <= 32767 - NS, len(nodes)
            pernodes.append(nodes)
            lut = {int(n): j + NS for j, n in enumerate(nodes)}
            for (s2, S, n_tiles, m) in program:
                if s2 != s:
                    continue
                lst = seg_info[c][s].get(S, [])
                for bi, (n, srcs) in enumerate(lst):
                    t, b = bi // m, bi % m
                    base = i0 + t * P + b * S
                    J[base] = lut[n]
                    for e, sv in enumerate(srcs):
                        J[base + 1 + e] = lut[int(sv)]
                nblocks = len(lst)
                for bi in range(nblocks, n_tiles * m):
                    t2, b2 = bi // m, bi % m
                    J[i0 + t2 * P + b2 * S + 1] = \
                        NS0 + int(rng.integers(0, NS1))
                t = 0
                while t < n_tiles:
                    nb = min(NBT, n_tiles - t)
                    ngc = (nb + GMAX - 1) // GMAX
                    k0 = 0
                    gslot = 0
                    while k0 < nb:
                        g = min(GMAX, nb - k0)
                        for k in range(g):
                            tabs = t + k0 + k
                            for b in range(m):
                                bi = tabs * m + b
                                if bi < nblocks:
                                    er.append(co + (k * m + b) * ngc + gslot)
                                    ec.append(k)
                                    en.append(lst[bi][0])
                        gslot += 1
                        k0 += g
                    co += P * ngc
                    t += nb
                i0 += n_tiles * P
        assert co == total_out_rows
        Js.append(J)
        grows.append(np.array(er, dtype=np.int64))
        gcols.append(np.array(ec, dtype=np.int64))
        gnodes.append(np.array(en, dtype=np.int64))
        seg_nodes.append(pernodes)
    return (program, total_tiles, total_out_rows, Js,
            grows, gcols, gnodes, seg_nodes)


def _build_masks(program):
    """Per-bucket masks: BM [P, m] (src slots), SM [P, P] (slot0 select),
    BMT [32, P] (BM transpose)."""
    keys = sorted({(S, m) for (_, S, _, m) in program})
    bm, sm, bmt, key_idx = [], [], [], {}
    for ki, (S, m) in enumerate(keys):
        B = np.zeros((P, m), dtype=np.float32)
        SEL = np.zeros((P, P), dtype=np.float32)
        BT = np.zeros((32, P), dtype=np.float32)
        for p in range(m * S):
            if p % S != 0:
                B[p, p // S] = 1.0
                BT[p // S, p] = 1.0
            SEL[(p // S) * S, p] = 1.0
        bm.append(B)
        sm.append(SEL)
        bmt.append(BT)
        key_idx[(S, m)] = ki
    bmc = np.concatenate(bm, 1).astype(np.float16)
    smc = np.concatenate(sm, 1).astype(np.float16)
    bmtc = np.concatenate(bmt, 1).astype(np.float16)
    bm_off = np.cumsum([0] + [b.shape[1] for b in bm])
    return bmc, smc, bmtc, bm_off, key_idx


def _build_program(program, total_tiles, total_out_rows, seg_rows, n_bm_cols,
                   nkeys):
    import os
    import concourse.bass as bass  # noqa: F401
    import concourse.tile as tile
    from concourse import bacc, mybir, library_config
    from concourse.mybir import ActivationFunctionType as AFT

    gather_only = os.environ.get("GAT_GATHER_ONLY") == "1"
    total_idx = total_tiles * P
    seg_base = np.cumsum([0] + list(seg_rows))
    F16 = mybir.dt.float16

    nc = bacc.Bacc("TRN2", target_bir_lowering=False,
                   num_swdge_queues=NQUEUES,
                   dynamic_dma_scratch_size=65536)
    XP = nc.dram_tensor("XP", [int(seg_base[-1]), ROW], F16,
                        kind="ExternalInput")
    IDX = nc.dram_tensor("IDX", [P, total_idx // 16], mybir.dt.int16,
                         kind="ExternalInput")
    W2R = nc.dram_tensor("W2R", [P, 2 * HIDDEN], F16, kind="ExternalInput")
    BM = nc.dram_tensor("BM", [P, n_bm_cols], F16, kind="ExternalInput")
    SM = nc.dram_tensor("SM", [P, P * nkeys], F16, kind="ExternalInput")
    BMT = nc.dram_tensor("BMT", [32, P * nkeys], F16, kind="ExternalInput")
    I128 = nc.dram_tensor("I128", [P, P], F16, kind="ExternalInput")
    OUT = nc.dram_tensor("OUT", [total_out_rows, GMAX * HIDDEN], F16,
                         kind="ExternalOutput")

    _, _, _, bm_off, key_idx = _build_masks(program)

    with tile.TileContext(nc) as tc:
        with (
            tc.tile_pool(name="msk", bufs=1) as mskp,
            tc.tile_pool(name="g", bufs=8) as gp,
            tc.tile_pool(name="tm", bufs=4) as tmp_p,
            tc.tile_pool(name="sc", bufs=6) as scp,
            tc.tile_pool(name="es", bufs=4) as esp,
            tc.tile_pool(name="st", bufs=4) as stp,
            tc.tile_pool(name="psc", bufs=4, space="PSUM") as pscp,
            tc.tile_pool(name="psU", bufs=4, space="PSUM") as psUp,
        ):
            nc.gpsimd.load_library(library_config.mlp)

            bmall = mskp.tile([P, n_bm_cols], F16)
            nc.sync.dma_start(bmall[:], BM[:])
            small = mskp.tile([P, P * nkeys], F16)
            nc.sync.dma_start(small[:], SM[:])
            bmtall = mskp.tile([32, P * nkeys], F16)
            nc.sync.dma_start(bmtall[:], BMT[:])
            i128 = mskp.tile([P, P], F16)
            nc.sync.dma_start(i128[:], I128[:])
            w2r = mskp.tile([P, 2 * HIDDEN], F16)
            nc.sync.dma_start(w2r[:], W2R[:])
            jtall = mskp.tile([P, total_idx // 16], mybir.dt.int16)
            nc.sync.dma_start(jtall[:], IDX[:])

            i0 = 0
            co = 0
            call = 0
            for (s, S, n_tiles, m) in program:
                ki = key_idx[(S, m)]
                t = 0
                while t < n_tiles:
                    nb = min(NBT, n_tiles - t)
                    nidx = nb * P
                    G = gp.tile([P, NBT, ROW], F16, tag="G")
                    nc.gpsimd.dma_gather(
                        out_ap=G[:, :nb, :],
                        in_ap=XP[int(seg_base[s]):int(seg_base[s + 1]), :],
                        idxs_ap=jtall[:, i0 // 16:(i0 + nidx) // 16],
                        num_idxs=nidx,
                        num_idxs_reg=nidx,
                        elem_size=ROW,
                        single_packet=False,
                        queue_num=call % NQUEUES,
                    )
                    call += 1
                    if gather_only:
                        co += P * ((nb + GMAX - 1) // GMAX)
                        i0 += nidx
                        t += nb
                        continue
                    # per-slot scores: sj, si via mul + free-dim reduce
                    tmp = tmp_p.tile([P, NBT, 2, HIDDEN], F16, tag="tmp")
                    nc.vector.tensor_mul(
                        tmp[:, :nb, :, :],
                        G[:, :nb, 0:HIDDEN].unsqueeze(2).broadcast_to(
                            [P, nb, 2, HIDDEN]),
                        w2r[:].rearrange("p (w e) -> p w e", w=2)
                        .unsqueeze(1).broadcast_to([P, nb, 2, HIDDEN]))
                    sco = scp.tile([P, NBT, 2], F16, tag="sco")
                    with nc.allow_low_precision(reason="fp16 scores |s|<8"):
                        nc.vector.tensor_reduce(
                            sco[:, :nb, :], tmp[:, :nb, :, :],
                            axis=mybir.AxisListType.X, op=mybir.AluOpType.add)
                    # raw score: psc0[p,k] = si[slot0(p)] + sj[p]
                    psc = pscp.tile([P, 3, NBT], mybir.dt.float32, tag="psc")
                    nc.tensor.matmul(
                        psc[:, 0, :nb],
                        lhsT=small[:, ki * P:(ki + 1) * P],
                        rhs=sco[:, :nb, 1],
                        start=True, stop=False)
                    nc.tensor.matmul(
                        psc[:, 0, :nb],
                        lhsT=i128[:],
                        rhs=sco[:, :nb, 0],
                        start=False, stop=True)
                    # leaky relu on DVE (keeps Act at Exp/Relu only)
                    esc = scp.tile([P, NBT], mybir.dt.float32, tag="esc")
                    nc.vector.tensor_scalar_mul(esc[:, :nb], psc[:, 0, :nb],
                                                LEAKY)
                    elr = scp.tile([P, NBT], F16, tag="elr")
                    nc.vector.tensor_max(elr[:, :nb], psc[:, 0, :nb],
                                         esc[:, :nb])
                    ex = scp.tile([P, NBT], F16, tag="ex")
                    nc.scalar.activation(ex[:, :nb], elr[:, :nb], AFT.Exp)
                    exm = scp.tile([P, NBT], F16, tag="exm")
                    nc.vector.tensor_mul(exm[:, :nb], ex[:, :nb],
                                         G[:, :nb, HIDDEN])
                    # denominators per block -> per-slot reciprocal
                    nc.tensor.matmul(
                        psc[0:m, 1, :nb],
                        lhsT=bmall[:, int(bm_off[ki]):int(bm_off[ki]) + m],
                        rhs=exm[:, :nb],
                        start=True, stop=True)
                    rec = scp.tile([32, NBT], F16, tag="rec")
                    with nc.allow_low_precision(
                            reason="fp16 1/denom; denom in [0.9, 9e4]"):
                        nc.vector.reciprocal(rec[:m, :nb],
                                             psc[0:m, 1, :nb])
                    nc.tensor.matmul(
                        psc[:, 2, :nb],
                        lhsT=bmtall[0:m, ki * P:(ki + 1) * P],
                        rhs=rec[:m, :nb],
                        start=True, stop=True)
                    exr = scp.tile([P, NBT], F16, tag="exr")
                    nc.vector.tensor_mul(exr[:, :nb], exm[:, :nb],
                                         psc[:, 2, :nb])
                    exsel = esp.tile([P, NBT, m], F16, tag="exsel")
                    nc.vector.tensor_mul(
                        exsel[:, :nb, :],
                        bmall[:, int(bm_off[ki]):int(bm_off[ki]) + m
                              ].unsqueeze(1).broadcast_to([P, nb, m]),
                        exr[:, :nb].unsqueeze(2).broadcast_to([P, nb, m]))
                    # aggregation groups
                    ngc = (nb + GMAX - 1) // GMAX
                    stage = stp.tile([P, GMAX, GMAX, HIDDEN], F16,
                                     tag="stage")
                    gslot = 0
                    k0 = 0
                    while k0 < nb:
                        g = min(GMAX, nb - k0)
                        psU = psUp.tile([P, GMAX, HIDDEN], mybir.dt.float32,
                                        tag="psU")
                        nc.tensor.matmul(
                            psU[0:g * m, 0:g, :],
                            lhsT=exsel[:, k0:k0 + g, :],
                            rhs=G[:, k0:k0 + g, 0:HIDDEN],
                            start=True, stop=True)
                        nc.scalar.activation(
                            stage[0:g * m, gslot, 0:g, :],
                            psU[0:g * m, 0:g, :], AFT.Relu)
                        gslot += 1
                        k0 += g
                    # row = co + p*ngc + gslot: contiguous per partition
                    nc.sync.dma_start(
                        OUT[co:co + P * ngc, :].rearrange(
                            "(p g) e -> p g e", g=ngc),
                        stage[:, :ngc, :, :])
                    co += P * ngc
                    i0 += nidx
                    t += nb
    nc.compile()
    return nc


def _install_profhook():
    """Register the axon NTFF profile hook (missing glue in this container)."""
    import contextlib
    import ctypes
    import sys
    import types

    if "antenv.axon_hooks" in sys.modules:
        return
    try:
        lib = ctypes.CDLL("/opt/axon/libaxon_pjrt.so")
        assert hasattr(lib, "axon_start_nrt_profile")
    except Exception:
        return
    lib.axon_start_nrt_profile.argtypes = [ctypes.POINTER(ctypes.c_int64),
                                           ctypes.c_size_t]
    lib.axon_start_nrt_profile.restype = ctypes.c_int64
    lib.axon_stop_nrt_profile.argtypes = [ctypes.c_char_p]
    lib.axon_stop_nrt_profile.restype = ctypes.c_int64

    @contextlib.contextmanager
    def _hook(output_dir, device_ids):
        import jax

        jax.devices()
        if device_ids:
            ids = (ctypes.c_int64 * len(device_ids))(*device_ids)
            rc = lib.axon_start_nrt_profile(ids, len(device_ids))
        else:
            rc = lib.axon_start_nrt_profile(None, 0)
        if rc != 0:
            raise RuntimeError(f"axon_start_nrt_profile rc={rc}")
        try:
            yield
        finally:
            lib.axon_stop_nrt_profile(str(output_dir).encode())

    mod = types.ModuleType("antenv.axon_hooks")
    mod.get_axon_ntff_profile_hook = lambda: _hook
    mod.set_axon_ntff_profile_hook = lambda h: None
    sys.modules["antenv.axon_hooks"] = mod
    import antenv

    antenv.axon_hooks = mod


def kernel(x, edge_index, w_i, w_j):
    import os
    from concourse.bass_utils import run_bass_kernel_spmd

    f16 = np.float16
    x = np.asarray(x, dtype=np.float32)
    edge_index = np.asarray(edge_index)
    w_i = np.asarray(w_i, dtype=np.float32)
    w_j = np.asarray(w_j, dtype=np.float32)
    n = x.shape[0]
    assert n == N_NODES and x.shape[1] == HIDDEN
    npc = n // N_CORES

    ej = edge_index[0].astype(np.int64)
    ei = edge_index[1].astype(np.int64)

    (program, total_tiles, total_out_rows, Js,
     grows, gcols, gnodes, seg_nodes) = \
        _build_layout(ej, ei, n, N_CORES)
    bmc, smc, bmtc, bm_off, key_idx = _build_masks(program)

    # per-segment table sizes uniform across cores (one shared program);
    # + NS0+NS1 sentinel rows
    NS = NS0 + NS1
    seg_rows = [NS + max(len(seg_nodes[c][s]) for c in range(N_CORES))
                for s in range(NSEG)]

    W2r = np.tile(np.concatenate([w_j, w_i]).astype(f16)[None, :], (P, 1))
    I128c = np.eye(P, dtype=f16)

    nc = _build_program(program, total_tiles, total_out_rows, seg_rows,
                        bmc.shape[1], len(key_idx))

    x16 = x.astype(f16)
    in_maps = []
    for c in range(N_CORES):
        tabs = []
        for s in range(NSEG):
            nodes = seg_nodes[c][s]
            t = np.zeros((seg_rows[s], ROW), dtype=f16)
            t[NS0:NS, HIDDEN] = 1.0  # flag-1 sentinels (zero x)
            t[NS:NS + len(nodes), 0:HIDDEN] = x16[nodes]
            t[NS:NS + len(nodes), HIDDEN] = 1.0
            tabs.append(t)
        XPc = np.ascontiguousarray(np.concatenate(tabs, 0))
        idxmat = np.ascontiguousarray(Js[c].reshape(-1, 16).T)
        IDXc = np.ascontiguousarray(np.tile(idxmat, (8, 1)))
        in_maps.append({
            "XP": XPc, "IDX": IDXc,
            "W2R": np.ascontiguousarray(W2r),
            "BM": np.ascontiguousarray(bmc),
            "SM": np.ascontiguousarray(smc),
            "BMT": np.ascontiguousarray(bmtc),
            "I128": np.ascontiguousarray(I128c),
        })
    trace = os.environ.get("GAT_TRACE") == "1"
    if trace:
        _install_profhook()
    res = run_bass_kernel_spmd(nc, in_maps, core_ids=list(range(N_CORES)),
                               trace=trace)
    if trace and res.exec_time_ns:
        print(f"HW exec time: {res.exec_time_ns} ns")

    out = np.zeros((n, HIDDEN), dtype=np.float32)
    for c in range(N_CORES):
        ot = res.results[c]["OUT"].reshape(total_out_rows, GMAX, HIDDEN)
        out[gnodes[c]] = ot[grows[c], gcols[c], :].astype(np.float32)
    return out


# revision 25
# speedup vs baseline: 5.1641x; 1.1637x over previous
"""GAT message-passing kernel for 8 TRN2 NeuronCores (Bass/Tile).

v5.1 strategy (dst-sharded, PE/Act-heavy softmax, no collectives):
  - Each core owns a contiguous range of destination nodes; the host routes
    each edge to the core owning its destination (edge_index[1]).  8 dst
    segments per core keep compact gather tables within int16 index space
    (last segment takes the remainder — v4 dropped the last 4 dsts/core).
  - Edges grouped per destination into blocks of S slots (slot 0 = the
    destination's own row, slots 1..deg = sources, rest point at the
    all-zero sentinel row 0).  Degrees pool into size classes so buckets
    stay large: less cross-core tile padding, fewer (bucket-bound) chunks.
    Table row fp16 [x(64) | flag | pad]; flag=0 on the sentinel masks the
    padded slots out of the softmax.
  - Per chunk of up to 16 tiles (128 slots each):
      dma_gather (4 SWDGE queues rotating, ~2.3ns/row) -> G [128, nb, 128]
      DVE: tmp = G.x * [wj|wi]; sco = reduce -> per-slot sj, si (fp16)
      PE:  psc = SM @ si + I128 @ sj        raw score (slot0 bcast + own sj)
      DVE: lrelu (mul + max);  Act: Exp -> ex fp16   (Act runs ONLY
           Exp/Relu — mixing in Lrelu/Copy thrashed the act table cache)
      DVE: exm = ex * flag;  PE: den = BM-contract(exm);  DVE: rec = 1/den
      PE:  recsel = BMT @ rec -> per-slot 1/denominator
      DVE: exr = exm * recsel; exsel = BM x exr      normalized alpha
      PE:  per <=4-tile group one matmul, stationary exsel [128, g*m],
           moving G.x [128, g, 64] -> psU [g*m, g, 64] (g-1 junk bands —
           junk costs no extra PE columns and the host skips it)
      Act: Relu psU -> fp16 stage
      one DMA per chunk -> OUT, row = base + p*ngc + group: each partition
      writes one contiguous run (the v5 (g p) order scattered 512B bursts)
  - Host extracts the diagonal bands from OUT and assembles the result.
"""
import numpy as np

N_NODES = 100000
HIDDEN = 64
N_CORES = 8
NSEG = 8                 # dst segments per core (int16 index headroom)
LEAKY = 0.01
P = 128
ROW = 128                # fp16 elements per table row (256B, dma_gather)
NBT = 16                 # tiles per gather chunk
GMAX = 4                 # tiles per aggregation group (4m <= 128)
NQUEUES = 4
NS0 = 256                # flag-0 sentinel rows (spread: avoids DRAM hotspot)
NS1 = 32                 # flag-1 sentinel rows (pad blocks, den=1)

# block slot-count classes (S = degree+1 rounded up to one of these).
# Chosen so m*S = (128//S)*S stays close to 128 (tile packing) while pad
# slots per block stay small where the degree mass is (Poisson ~16).
SCLASSES = [2, 3, 4, 5, 6, 7, 8, 9, 10, 11, 12, 13, 14, 16, 18, 21, 25,
            32, 42, 63, 127]


def _sclass(d):
    S = d + 1
    for sc in SCLASSES:
        if sc >= S:
            return sc
    raise ValueError(f"unsupported degree {d}")


def _build_layout(ej, ei, n_nodes, ncores):
    """Per-core, per-segment compact tables + class-bucket/chunk structure.

    Destinations are dealt to (core, segment) round-robin by degree rank so
    every (core, seg, class) bucket has a near-identical block count — the
    shared program pads each bucket to the max over cores, so contiguous
    range sharding wasted ~18% of the gather rows on padding.
    """
    deg = np.bincount(ei, minlength=n_nodes)
    order = np.argsort(ei, kind="stable")
    src_sorted = ej[order]
    starts = np.concatenate([[0], np.cumsum(deg)])
    nz = np.where(deg > 0)[0]
    rank = nz[np.argsort(-deg[nz], kind="stable")]
    # rank r -> core r%ncores, segment (r//ncores)%NSEG
    seg_info = [[{} for _ in range(NSEG)] for _ in range(ncores)]
    for r, n in enumerate(rank):
        c = r % ncores
        s = (r // ncores) % NSEG
        d = int(deg[n])
        seg_info[c][s].setdefault(_sclass(d), []).append(
            (int(n), src_sorted[starts[n]:starts[n + 1]]))

    # shared bucket structure: n_tiles = max over cores
    program = []  # (seg, S, n_tiles, m)
    for s in range(NSEG):
        all_S = sorted({S for c in range(ncores)
                        for S in seg_info[c][s].keys()})
        for S in all_S:
            m = min(P // S, 32)
            maxb = max(len(seg_info[c][s].get(S, [])) for c in range(ncores))
            n_tiles = (maxb + m - 1) // m
            program.append((s, S, n_tiles, m))

    total_tiles = sum(p[2] for p in program)
    total_idx = total_tiles * P

    # chunk walk (shared): chunk list of (s, S, m, nb, ngc); groups g<=GMAX
    chunks = []
    for (s, S, n_tiles, m) in program:
        t = 0
        while t < n_tiles:
            nb = min(NBT, n_tiles - t)
            ngc = (nb + GMAX - 1) // GMAX
            chunks.append((s, S, m, nb, ngc))
            t += nb
    total_out_rows = sum(P * ch[4] for ch in chunks)

    # per-core: index streams + output extraction maps
    rng = np.random.default_rng(1234)
    NS = NS0 + NS1
    Js = []
    grows, gcols, gnodes = [], [], []
    seg_nodes = []  # [core][seg] -> global node ids (row NS+r = nodes[r])
    for c in range(ncores):
        J = rng.integers(0, NS0, size=total_idx).astype(np.int16)
        er, ec, en = [], [], []
        pernodes = []
        i0 = 0
        co = 0  # OUT row base of current chunk
        for s in range(NSEG):
            allref = [srcs for S, lst in seg_info[c][s].items()
                      for (_, srcs) in lst]
            allref.append(np.array(
                [n for S, lst in seg_info[c][s].items()
                 for (n, _) in lst], dtype=np.int64))
            nodes = np.unique(np.concatenate(allref)) if allref else \
                np.empty(0, dtype=np.int64)
            assert len(nodes) <= 32767 - NS, len(nodes)
            pernodes.append(nodes)
            lut = {int(n): j + NS for j, n in enumerate(nodes)}
            for (s2, S, n_tiles, m) in program:
                if s2 != s:
                    continue
                lst = seg_info[c][s].get(S, [])
                for bi, (n, srcs) in enumerate(lst):
                    t, b = bi // m, bi % m
                    base = i0 + t * P + b * S
                    J[base] = lut[n]
                    for e, sv in enumerate(srcs):
                        J[base + 1 + e] = lut[int(sv)]
                nblocks = len(lst)
                for bi in range(nblocks, n_tiles * m):
                    t2, b2 = bi // m, bi % m
                    J[i0 + t2 * P + b2 * S + 1] = \
                        NS0 + int(rng.integers(0, NS1))
                t = 0
                while t < n_tiles:
                    nb = min(NBT, n_tiles - t)
                    ngc = (nb + GMAX - 1) // GMAX
                    k0 = 0
                    gslot = 0
                    while k0 < nb:
                        g = min(GMAX, nb - k0)
                        for k in range(g):
                            tabs = t + k0 + k
                            for b in range(m):
                                bi = tabs * m + b
                                if bi < nblocks:
                                    er.append(co + (k * m + b) * ngc + gslot)
                                    ec.append(k)
                                    en.append(lst[bi][0])
                        gslot += 1
                        k0 += g
                    co += P * ngc
                    t += nb
                i0 += n_tiles * P
        assert co == total_out_rows
        Js.append(J)
        grows.append(np.array(er, dtype=np.int64))
        gcols.append(np.array(ec, dtype=np.int64))
        gnodes.append(np.array(en, dtype=np.int64))
        seg_nodes.append(pernodes)
    return (program, total_tiles, total_out_rows, Js,
            grows, gcols, gnodes, seg_nodes)


def _build_masks(program):
    """Per-bucket masks: BM [P, m] (src slots), SM [P, P] (slot0 select),
    BMT [32, P] (BM transpose)."""
    keys = sorted({(S, m) for (_, S, _, m) in program})
    bm, sm, bmt, key_idx = [], [], [], {}
    for ki, (S, m) in enumerate(keys):
        B = np.zeros((P, m), dtype=np.float32)
        SEL = np.zeros((P, P), dtype=np.float32)
        BT = np.zeros((32, P), dtype=np.float32)
        for p in range(m * S):
            if p % S != 0:
                B[p, p // S] = 1.0
                BT[p // S, p] = 1.0
            SEL[(p // S) * S, p] = 1.0
        bm.append(B)
        sm.append(SEL)
        bmt.append(BT)
        key_idx[(S, m)] = ki
    bmc = np.concatenate(bm, 1).astype(np.float16)
    smc = np.concatenate(sm, 1).astype(np.float16)
    bmtc = np.concatenate(bmt, 1).astype(np.float16)
    bm_off = np.cumsum([0] + [b.shape[1] for b in bm])
    return bmc, smc, bmtc, bm_off, key_idx


def _build_program(program, total_tiles, total_out_rows, seg_rows, n_bm_cols,
                   nkeys):
    import os
    import concourse.bass as bass  # noqa: F401
    import concourse.tile as tile
    from concourse import bacc, mybir, library_config
    from concourse.mybir import ActivationFunctionType as AFT

    gather_only = os.environ.get("GAT_GATHER_ONLY") == "1"
    total_idx = total_tiles * P
    seg_base = np.cumsum([0] + list(seg_rows))
    F16 = mybir.dt.float16

    nc = bacc.Bacc("TRN2", target_bir_lowering=False,
                   num_swdge_queues=NQUEUES,
                   dynamic_dma_scratch_size=65536)
    XP = nc.dram_tensor("XP", [int(seg_base[-1]), ROW], F16,
                        kind="ExternalInput")
    IDX = nc.dram_tensor("IDX", [P, total_idx // 16], mybir.dt.int16,
                         kind="ExternalInput")
    W2R = nc.dram_tensor("W2R", [P, 2 * HIDDEN], F16, kind="ExternalInput")
    BM = nc.dram_tensor("BM", [P, n_bm_cols], F16, kind="ExternalInput")
    SM = nc.dram_tensor("SM", [P, P * nkeys], F16, kind="ExternalInput")
    BMT = nc.dram_tensor("BMT", [32, P * nkeys], F16, kind="ExternalInput")
    I128 = nc.dram_tensor("I128", [P, P], F16, kind="ExternalInput")
    OUT = nc.dram_tensor("OUT", [total_out_rows, GMAX * HIDDEN], F16,
                         kind="ExternalOutput")

    _, _, _, bm_off, key_idx = _build_masks(program)

    with tile.TileContext(nc) as tc:
        with (
            tc.tile_pool(name="msk", bufs=1) as mskp,
            tc.tile_pool(name="g", bufs=10) as gp,
            tc.tile_pool(name="tm", bufs=3) as tmp_p,
            tc.tile_pool(name="sc", bufs=6) as scp,
            tc.tile_pool(name="es", bufs=4) as esp,
            tc.tile_pool(name="st", bufs=4) as stp,
            tc.tile_pool(name="psc", bufs=4, space="PSUM") as pscp,
            tc.tile_pool(name="psU", bufs=4, space="PSUM") as psUp,
        ):
            nc.gpsimd.load_library(library_config.mlp)

            bmall = mskp.tile([P, n_bm_cols], F16)
            nc.sync.dma_start(bmall[:], BM[:])
            small = mskp.tile([P, P * nkeys], F16)
            nc.sync.dma_start(small[:], SM[:])
            bmtall = mskp.tile([32, P * nkeys], F16)
            nc.sync.dma_start(bmtall[:], BMT[:])
            i128 = mskp.tile([P, P], F16)
            nc.sync.dma_start(i128[:], I128[:])
            w2r = mskp.tile([P, 2 * HIDDEN], F16)
            nc.sync.dma_start(w2r[:], W2R[:])
            jtall = mskp.tile([P, total_idx // 16], mybir.dt.int16)
            nc.sync.dma_start(jtall[:], IDX[:])

            i0 = 0
            co = 0
            call = 0
            for (s, S, n_tiles, m) in program:
                ki = key_idx[(S, m)]
                t = 0
                while t < n_tiles:
                    nb = min(NBT, n_tiles - t)
                    nidx = nb * P
                    G = gp.tile([P, NBT, ROW], F16, tag="G")
                    nc.gpsimd.dma_gather(
                        out_ap=G[:, :nb, :],
                        in_ap=XP[int(seg_base[s]):int(seg_base[s + 1]), :],
                        idxs_ap=jtall[:, i0 // 16:(i0 + nidx) // 16],
                        num_idxs=nidx,
                        num_idxs_reg=nidx,
                        elem_size=ROW,
                        single_packet=False,
                        queue_num=call % NQUEUES,
                    )
                    call += 1
                    if gather_only:
                        co += P * ((nb + GMAX - 1) // GMAX)
                        i0 += nidx
                        t += nb
                        continue
                    # per-slot scores: sj, si via mul + free-dim reduce
                    tmp = tmp_p.tile([P, NBT, 2, HIDDEN], F16, tag="tmp")
                    nc.vector.tensor_mul(
                        tmp[:, :nb, :, :],
                        G[:, :nb, 0:HIDDEN].unsqueeze(2).broadcast_to(
                            [P, nb, 2, HIDDEN]),
                        w2r[:].rearrange("p (w e) -> p w e", w=2)
                        .unsqueeze(1).broadcast_to([P, nb, 2, HIDDEN]))
                    sco = scp.tile([P, NBT, 2], F16, tag="sco")
                    with nc.allow_low_precision(reason="fp16 scores |s|<8"):
                        nc.vector.tensor_reduce(
                            sco[:, :nb, :], tmp[:, :nb, :, :],
                            axis=mybir.AxisListType.X, op=mybir.AluOpType.add)
                    # raw score: psc0[p,k] = si[slot0(p)] + sj[p]
                    psc = pscp.tile([P, 3, NBT], mybir.dt.float32, tag="psc")
                    nc.tensor.matmul(
                        psc[:, 0, :nb],
                        lhsT=small[:, ki * P:(ki + 1) * P],
                        rhs=sco[:, :nb, 1],
                        start=True, stop=False)
                    nc.tensor.matmul(
                        psc[:, 0, :nb],
                        lhsT=i128[:],
                        rhs=sco[:, :nb, 0],
                        start=False, stop=True)
                    # leaky relu on DVE (keeps Act at Exp/Relu only)
                    esc = scp.tile([P, NBT], mybir.dt.float32, tag="esc")
                    nc.vector.tensor_scalar_mul(esc[:, :nb], psc[:, 0, :nb],
                                                LEAKY)
                    elr = scp.tile([P, NBT], F16, tag="elr")
                    nc.vector.tensor_max(elr[:, :nb], psc[:, 0, :nb],
                                         esc[:, :nb])
                    ex = scp.tile([P, NBT], F16, tag="ex")
                    nc.scalar.activation(ex[:, :nb], elr[:, :nb], AFT.Exp)
                    exm = scp.tile([P, NBT], F16, tag="exm")
                    nc.vector.tensor_mul(exm[:, :nb], ex[:, :nb],
                                         G[:, :nb, HIDDEN])
                    # denominators per block -> per-slot reciprocal
                    nc.tensor.matmul(
                        psc[0:m, 1, :nb],
                        lhsT=bmall[:, int(bm_off[ki]):int(bm_off[ki]) + m],
                        rhs=exm[:, :nb],
                        start=True, stop=True)
                    rec = scp.tile([32, NBT], F16, tag="rec")
                    with nc.allow_low_precision(
                            reason="fp16 1/denom; denom in [0.9, 9e4]"):
                        nc.vector.reciprocal(rec[:m, :nb],
                                             psc[0:m, 1, :nb])
                    nc.tensor.matmul(
                        psc[:, 2, :nb],
                        lhsT=bmtall[0:m, ki * P:(ki + 1) * P],
                        rhs=rec[:m, :nb],
                        start=True, stop=True)
                    exr = scp.tile([P, NBT], F16, tag="exr")
                    nc.vector.tensor_mul(exr[:, :nb], exm[:, :nb],
                                         psc[:, 2, :nb])
                    exsel = esp.tile([P, NBT, m], F16, tag="exsel")
                    nc.vector.tensor_mul(
                        exsel[:, :nb, :],
                        bmall[:, int(bm_off[ki]):int(bm_off[ki]) + m
                              ].unsqueeze(1).broadcast_to([P, nb, m]),
                        exr[:, :nb].unsqueeze(2).broadcast_to([P, nb, m]))
                    # aggregation groups
                    ngc = (nb + GMAX - 1) // GMAX
                    stage = stp.tile([P, GMAX, GMAX, HIDDEN], F16,
                                     tag="stage")
                    gslot = 0
                    k0 = 0
                    while k0 < nb:
                        g = min(GMAX, nb - k0)
                        psU = psUp.tile([P, GMAX, HIDDEN], mybir.dt.float32,
                                        tag="psU")
                        nc.tensor.matmul(
                            psU[0:g * m, 0:g, :],
                            lhsT=exsel[:, k0:k0 + g, :],
                            rhs=G[:, k0:k0 + g, 0:HIDDEN],
                            start=True, stop=True)
                        nc.scalar.activation(
                            stage[0:g * m, gslot, 0:g, :],
                            psU[0:g * m, 0:g, :], AFT.Relu)
                        gslot += 1
                        k0 += g
                    # row = co + p*ngc + gslot: contiguous per partition
                    nc.sync.dma_start(
                        OUT[co:co + P * ngc, :].rearrange(
                            "(p g) e -> p g e", g=ngc),
                        stage[:, :ngc, :, :])
                    co += P * ngc
                    i0 += nidx
                    t += nb
    nc.compile()
    return nc


def _install_profhook():
    """Register the axon NTFF profile hook (missing glue in this container)."""
    import contextlib
    import ctypes
    import sys
    import types

    if "antenv.axon_hooks" in sys.modules:
        return
    try:
        lib = ctypes.CDLL("/opt/axon/libaxon_pjrt.so")
        assert hasattr(lib, "axon_start_nrt_profile")
    except Exception:
        return
    lib.axon_start_nrt_profile.argtypes = [ctypes.POINTER(ctypes.c_int64),
                                           ctypes.c_size_t]
    lib.axon_start_nrt_profile.restype = ctypes.c_int64
    lib.axon_stop_nrt_profile.argtypes = [ctypes.c_char_p]
    lib.axon_stop_nrt_profile.restype = ctypes.c_int64

    @contextlib.contextmanager
    def _hook(output_dir, device_ids):
        import jax

        jax.devices()
        if device_ids:
            ids = (ctypes.c_int64 * len(device_ids))(*device_ids)
            rc = lib.axon_start_nrt_profile(ids, len(device_ids))
        else:
            rc = lib.axon_start_nrt_profile(None, 0)
        if rc != 0:
            raise RuntimeError(f"axon_start_nrt_profile rc={rc}")
        try:
            yield
        finally:
            lib.axon_stop_nrt_profile(str(output_dir).encode())

    mod = types.ModuleType("antenv.axon_hooks")
    mod.get_axon_ntff_profile_hook = lambda: _hook
    mod.set_axon_ntff_profile_hook = lambda h: None
    sys.modules["antenv.axon_hooks"] = mod
    import antenv

    antenv.axon_hooks = mod


def kernel(x, edge_index, w_i, w_j):
    import os
    from concourse.bass_utils import run_bass_kernel_spmd

    f16 = np.float16
    x = np.asarray(x, dtype=np.float32)
    edge_index = np.asarray(edge_index)
    w_i = np.asarray(w_i, dtype=np.float32)
    w_j = np.asarray(w_j, dtype=np.float32)
    n = x.shape[0]
    assert n == N_NODES and x.shape[1] == HIDDEN
    npc = n // N_CORES

    ej = edge_index[0].astype(np.int64)
    ei = edge_index[1].astype(np.int64)

    (program, total_tiles, total_out_rows, Js,
     grows, gcols, gnodes, seg_nodes) = \
        _build_layout(ej, ei, n, N_CORES)
    bmc, smc, bmtc, bm_off, key_idx = _build_masks(program)

    # per-segment table sizes uniform across cores (one shared program);
    # + NS0+NS1 sentinel rows
    NS = NS0 + NS1
    seg_rows = [NS + max(len(seg_nodes[c][s]) for c in range(N_CORES))
                for s in range(NSEG)]

    W2r = np.tile(np.concatenate([w_j, w_i]).astype(f16)[None, :], (P, 1))
    I128c = np.eye(P, dtype=f16)

    nc = _build_program(program, total_tiles, total_out_rows, seg_rows,
                        bmc.shape[1], len(key_idx))

    x16 = x.astype(f16)
    in_maps = []
    for c in range(N_CORES):
        tabs = []
        for s in range(NSEG):
            nodes = seg_nodes[c][s]
            t = np.zeros((seg_rows[s], ROW), dtype=f16)
            t[NS0:NS, HIDDEN] = 1.0  # flag-1 sentinels (zero x)
            t[NS:NS + len(nodes), 0:HIDDEN] = x16[nodes]
            t[NS:NS + len(nodes), HIDDEN] = 1.0
            tabs.append(t)
        XPc = np.ascontiguousarray(np.concatenate(tabs, 0))
        idxmat = np.ascontiguousarray(Js[c].reshape(-1, 16).T)
        IDXc = np.ascontiguousarray(np.tile(idxmat, (8, 1)))
        in_maps.append({
            "XP": XPc, "IDX": IDXc,
            "W2R": np.ascontiguousarray(W2r),
            "BM": np.ascontiguousarray(bmc),
            "SM": np.ascontiguousarray(smc),
            "BMT": np.ascontiguousarray(bmtc),
            "I128": np.ascontiguousarray(I128c),
        })
    trace = os.environ.get("GAT_TRACE") == "1"
    if trace:
        _install_profhook()
    res = run_bass_kernel_spmd(nc, in_maps, core_ids=list(range(N_CORES)),
                               trace=trace)
    if trace and res.exec_time_ns:
        print(f"HW exec time: {res.exec_time_ns} ns")

    out = np.zeros((n, HIDDEN), dtype=np.float32)
    for c in range(N_CORES):
        ot = res.results[c]["OUT"].reshape(total_out_rows, GMAX, HIDDEN)
        out[gnodes[c]] = ot[grows[c], gcols[c], :].astype(np.float32)
    return out
